# revision 13
# baseline (speedup 1.0000x reference)
"""RWKV6 block (nn_Block_14602888806424) on 8 Trainium2 NeuronCores.

Token-sharded (sequence-parallel): each core owns 512 tokens (B=2 x 4
blocks); matmuls/LNs/mixing are token-local in channel-major layout.
r/k/w/v are redistributed head-sharded via AllToAll around the chunked
(L=128) WKV linear-attention scan (4 heads/core); a second pair of
AllToAlls (split by head-pair half) returns raw y; GroupNorm is applied
in the token-sharded domain with PE-based group stats. A small AllGather
carries the 1-token boundary halo for the second token-shift.

Perf structure: all collective payloads are bf16; the forward exchange is
split (v first, then r/k/w even channel-halves, then odd) so projection
matmuls overlap collective time; DMA loads that wait on collective
results issue from the ACT HWDGE queue so they cannot head-of-line-block
the SP DMA queue; weights are host-pre-tiled so every streamed weight
tile is a contiguous per-partition DMA; matmul operands are bf16 (fp32
matmuls run at 1/4 rate).
"""

import sys
import numpy as np

sys.path.insert(0, "/opt/trn_rl_repo")

import concourse.bass as bass
import concourse.bacc as bacc
import concourse.mybir as mybir
import concourse.tile as tile
from concourse import bass_utils

F32 = mybir.dt.float32
BF16 = mybir.dt.bfloat16
NP_BF16 = mybir.dt.np(BF16)
AF = mybir.ActivationFunctionType
ALU = mybir.AluOpType

B, T, C, H, N, FF = 2, 2048, 2048, 32, 64, 7168
D_MIX, D_DECAY = 32, 64
EPS_LN = 1e-5
EPS_LNX = 1e-5 * 8.0**2
NCORE = 8
TB = 512
KC = C // 128          # 16
KF = FF // 128         # 56
LCH = 256              # channels per core (4 heads)
RG = [list(range(NCORE))]


def build_program():
    nc = bacc.Bacc("TRN2", target_bir_lowering=False, debug=False,
                   num_devices=NCORE, enable_asserts=False)

    def din(name, shape, dt=F32):
        return nc.dram_tensor(name, list(shape), dt, kind="ExternalInput").ap()

    xT = din("xT", (C, TB + 1))
    halo_mask = din("halo_mask", (128, 1))
    sel_prev = din("sel_prev", (NCORE, 1), BF16)
    u_loc = din("u_loc", (128, 2))
    lnx_wb = din("lnx_wb", (C, 2))
    ln1_wb = din("ln1_wb", (C, 2))
    ln2_wb = din("ln2_wb", (C, 2))
    tm_maaT = din("tm_maaT", (C, 6))
    cm_maaT = din("cm_maaT", (C, 2))
    td_col = din("td_col", (C, 1))
    ident = din("ident", (128, 128), BF16)
    mask_su = din("mask_su", (128, 128), BF16)
    gsel = din("gsel", (128, KC * 32), BF16)
    bsel = din("bsel", (32, KC * 128), BF16)
    maa_w1 = din("maa_w1", (C, 5 * D_MIX), BF16)
    maa_w2p = din("maa_w2p", (32, 5, KC, 128), BF16)
    td_w1 = din("td_w1", (C, D_DECAY), BF16)
    td_w2p = din("td_w2p", (64, KC, 128), BF16)
    Wp = {k: din(f"W{k}_p", (KC, 128, KC, 128), BF16)
          for k in ["r", "k", "g", "o", "cr"]}
    Wv = din("Wv", (C, C), BF16)
    Wck_p = din("Wck_p", (KF, 128, KC, 128), BF16)
    Wcv_p = din("Wcv_p", (KC, 128, KF, 128), BF16)

    outT = nc.dram_tensor("out", [C, TB], F32, kind="ExternalOutput").ap()

    with tile.TileContext(nc) as tc:
        import contextlib
        with contextlib.ExitStack() as ctx:
            dram = ctx.enter_context(tc.tile_pool(name="dram", bufs=1,
                                                  space="DRAM"))
            cpool = ctx.enter_context(tc.tile_pool(name="const", bufs=1))
            big = ctx.enter_context(tc.tile_pool(name="big", bufs=1))
            wstr = ctx.enter_context(tc.tile_pool(name="wstr", bufs=3))
            sc = ctx.enter_context(tc.tile_pool(name="scratch", bufs=2))
            scw = ctx.enter_context(tc.tile_pool(name="scw", bufs=1))
            lnp = ctx.enter_context(tc.tile_pool(name="lnp", bufs=1))
            ps = ctx.enter_context(
                tc.tile_pool(name="psum", bufs=8, space="PSUM"))

            def pp(p_, f_):
                return ps.tile([p_, f_], F32, tag="pp", name="pp")

            def ppb(p_, f_):
                return ps.tile([p_, f_], BF16, tag="pp", name="ppb")

            # ---- DRAM internals (all collective payloads bf16) ----
            a2aA_in = dram.tile([NCORE, 3, 128, TB], BF16, tag="a2aA_in")
            a2aA_out = dram.tile([NCORE, 3, 128, TB], BF16, tag="a2aA_out")
            a2aB_in = dram.tile([NCORE, 3, 128, TB], BF16, tag="a2aB_in")
            a2aB_out = dram.tile([NCORE, 3, 128, TB], BF16, tag="a2aB_out")
            a2v_in = dram.tile([NCORE, TB, LCH], BF16, tag="a2v_in")
            a2v_out = dram.tile([NCORE, TB, LCH], BF16, tag="a2v_out")
            a2b0_in = dram.tile([NCORE, 128, TB], BF16, tag="a2b0_in")
            a2b0_out = dram.tile([NCORE, 128, TB], BF16, tag="a2b0_out")
            a2b1_in = dram.tile([NCORE, 128, TB], BF16, tag="a2b1_in")
            a2b1_out = dram.tile([NCORE, 128, TB], BF16, tag="a2b1_out")
            ag_in = dram.tile([1, C], BF16, tag="ag_in")
            ag_out = dram.tile([NCORE, C], BF16, tag="ag_out",
                               addr_space="Shared")
            x2d = dram.tile([C, TB], F32, tag="x2d")

            # ---- constants ----
            def cload(name, src, shape, dt=F32, rearr=None):
                t = cpool.tile(list(shape), dt, tag=name)
                nc.sync.dma_start(t[:], src if rearr is None
                                  else src.rearrange(rearr, p=128))
                return t

            c_ln1 = cload("c_ln1", ln1_wb, (128, KC, 2), F32, "(k p) f -> p k f")
            c_ln2 = cload("c_ln2", ln2_wb, (128, KC, 2), F32, "(k p) f -> p k f")
            c_lnx = cload("c_lnx", lnx_wb, (128, KC, 2), F32, "(k p) f -> p k f")
            c_tm = cload("c_tm", tm_maaT, (128, KC, 6), F32, "(k p) f -> p k f")
            c_cm = cload("c_cm", cm_maaT, (128, KC, 2), F32, "(k p) f -> p k f")
            c_td = cload("c_td", td_col, (128, KC, 1), F32, "(k p) f -> p k f")
            c_hm = cload("c_hm", halo_mask, (128, 1))
            c_sel = cload("c_sel", sel_prev, (NCORE, 1), BF16)
            c_u = cload("c_u", u_loc, (128, 2))
            c_id = cload("c_id", ident, (128, 128), BF16)
            c_msk = cload("c_msk", mask_su, (128, 128), BF16)
            c_gsel = cload("c_gsel", gsel, (128, KC, 32), BF16)
            c_bsel = cload("c_bsel", bsel, (32, KC, 128), BF16)
            c_w1 = cload("c_w1", maa_w1, (128, KC, 5 * D_MIX), BF16,
                         "(k p) f -> p k f")
            c_td1 = cload("c_td1", td_w1, (128, KC, D_DECAY), BF16,
                          "(k p) f -> p k f")
            c_td2 = cload("c_td2", td_w2p, (64, KC, 128), BF16)
            ones_col = cpool.tile([128, 1], BF16, tag="ones_col")
            nc.vector.memset(ones_col[:], 1.0)
            ones_row = cpool.tile([1, 128], BF16, tag="ones_row")
            nc.vector.memset(ones_row[:], 1.0)
            for _cv in (EPS_LN, EPS_LNX):
                cvt = cpool.tile([128, 1], F32, tag=f"cv{_cv}", name="cvt")
                nc.vector.memset(cvt[:], _cv)
                nc.const_aps.aps[(F32, _cv)] = cvt[:]

            # ---- persistent SBUF ----
            xb = big.tile([128, KC, TB + 1], BF16, tag="xb")
            ht = big.tile([128, KC, TB + 1], BF16, tag="ht")
            xx = big.tile([128, KC, TB], BF16, tag="xx")      # later xk2
            gsb = big.tile([128, KC, TB], BF16, tag="gsb")    # later xr2

            # ============ LN1 over TB+1 cols (src resident in xb) ============
            psA, psB = pp(1, TB), pp(1, 1)
            psA2, psB2 = pp(1, TB), pp(1, 1)
            for k in range(KC):
                t = sc.tile([128, TB + 1], F32, tag="e1", bufs=2)
                nc.sync.dma_start(t[:], xT[128 * k:128 * (k + 1), :])
                nc.scalar.activation(xb[:, k, :], t[:], AF.Copy)
                sq = sc.tile([128, TB + 1], BF16, tag="e2")
                nc.scalar.activation(sq[:], xb[:, k, :], AF.Square)
                st, sp = (k == 0), (k == KC - 1)
                nc.tensor.matmul(psA[:], ones_col[:], xb[:, k, 0:TB],
                                 start=st, stop=sp)
                nc.tensor.matmul(psB[:], ones_col[:], xb[:, k, TB:TB + 1],
                                 start=st, stop=sp)
                nc.tensor.matmul(psA2[:], ones_col[:], sq[:, 0:TB],
                                 start=st, stop=sp)
                nc.tensor.matmul(psB2[:], ones_col[:], sq[:, TB:TB + 1],
                                 start=st, stop=sp)
            stats = lnp.tile([1, 2 * (TB + 1)], F32, tag="ln_stats")
            mean, msq = stats[:, 0:TB + 1], stats[:, TB + 1:]
            nc.scalar.activation(mean[:, 0:TB], psA[:], AF.Copy, scale=1.0 / C)
            nc.scalar.activation(mean[:, TB:TB + 1], psB[:], AF.Copy,
                                 scale=1.0 / C)
            nc.scalar.activation(msq[:, 0:TB], psA2[:], AF.Copy, scale=1.0 / C)
            nc.scalar.activation(msq[:, TB:TB + 1], psB2[:], AF.Copy,
                                 scale=1.0 / C)
            wk = lnp.tile([1, TB + 1], F32, tag="ln_work")
            nc.vector.tensor_mul(wk[:], mean[:], mean[:])
            nc.vector.tensor_sub(wk[:], msq[:], wk[:])
            nc.scalar.activation(wk[:], wk[:], AF.Sqrt, bias=EPS_LN)
            nc.vector.reciprocal(wk[:], wk[:])
            stb = lnp.tile([1, 2 * (TB + 1)], BF16, tag="ln_stb")
            nc.vector.tensor_copy(stb[:, 0:TB + 1], mean[:])
            nc.vector.tensor_copy(stb[:, TB + 1:], wk[:])
            bmp, bmp2 = pp(128, TB), pp(128, 1)
            bip, bip2 = pp(128, TB), pp(128, 1)
            nc.tensor.matmul(bmp[:], ones_row[:], stb[:, 0:TB],
                             start=True, stop=True)
            nc.tensor.matmul(bmp2[:], ones_row[:], stb[:, TB:TB + 1],
                             start=True, stop=True)
            nc.tensor.matmul(bip[:], ones_row[:], stb[:, TB + 1:2 * TB + 1],
                             start=True, stop=True)
            nc.tensor.matmul(bip2[:], ones_row[:], stb[:, 2 * TB + 1:],
                             start=True, stop=True)
            bc = lnp.tile([128, 2 * (TB + 1)], BF16, tag="ln_bc")
            bm, bi = bc[:, 0:TB + 1], bc[:, TB + 1:]
            nc.vector.tensor_copy(bm[:, 0:TB], bmp[:])
            nc.vector.tensor_copy(bm[:, TB:TB + 1], bmp2[:])
            nc.vector.tensor_copy(bi[:, 0:TB], bip[:])
            nc.vector.tensor_copy(bi[:, TB:TB + 1], bip2[:])
            for k in range(KC):
                tn = sc.tile([128, TB + 1], BF16, tag="e2")
                nc.vector.tensor_sub(tn[:], xb[:, k, :], bm[:])
                nc.vector.tensor_mul(tn[:], tn[:], bi[:])
                d = ht[:, k, :]
                nc.vector.tensor_scalar(d, tn[:], c_ln1[:, k, 0:1],
                                        c_ln1[:, k, 1:2], ALU.mult, ALU.add)
                nc.vector.tensor_scalar(d[:, 0:1], d[:, 0:1], c_hm[:],
                                        None, ALU.mult)
                nc.vector.tensor_sub(xx[:, k, :], ht[:, k, 0:TB],
                                     ht[:, k, 1:TB + 1])

            # ============ maa ============
            aps1, aps2 = pp(128, TB), pp(32, TB)
            for k in range(KC):
                xxx = sc.tile([128, TB], BF16, tag="xxx")
                nc.vector.scalar_tensor_tensor(
                    xxx[:], xx[:, k, :], c_tm[:, k, 0:1], ht[:, k, 1:TB + 1],
                    ALU.mult, ALU.add)
                nc.tensor.matmul(aps1[:], c_w1[:, k, 0:128], xxx[:],
                                 start=(k == 0), stop=(k == KC - 1))
                nc.tensor.matmul(aps2[:], c_w1[:, k, 128:160], xxx[:],
                                 start=(k == 0), stop=(k == KC - 1))
            aTs = [cpool.tile([32, TB], BF16, tag=f"aT{i}", name="aTs")
                   for i in range(5)]
            for i in range(4):
                nc.scalar.activation(aTs[i][:], aps1[32 * i:32 * (i + 1), :],
                                     AF.Tanh)
            nc.scalar.activation(aTs[4][:], aps2[0:32, :], AF.Tanh)

            def make_mix(i, tag):
                mt = big.tile([128, KC, TB], BF16, tag=tag, name="mixbuf")
                for k in range(KC):
                    w2s = wstr.tile([32, 128], BF16, tag="w2s")
                    nc.sync.dma_start(w2s[:], maa_w2p[:, i, k, :])
                    mp = pp(128, TB)
                    nc.tensor.matmul(mp[:], w2s[:], aTs[i][:],
                                     start=True, stop=True)
                    t = sc.tile([128, TB], BF16, tag="g1")
                    nc.vector.scalar_tensor_tensor(
                        t[:], mp[:], c_tm[:, k, i + 1:i + 2], xx[:, k, :],
                        ALU.add, ALU.mult)
                    nc.vector.tensor_add(mt[:, k, :], t[:],
                                         ht[:, k, 1:TB + 1])
                return mt

            def proj_cm(wp_ap, sink, src_view, ms):
                for m in ms:
                    wt = wstr.tile([128, KC, 128], BF16, tag="wstream", bufs=2)
                    nc.sync.dma_start(wt[:], wp_ap[m])
                    pt = pp(128, TB)
                    for k in range(KC):
                        nc.tensor.matmul(pt[:], wt[:, k, :], src_view(k),
                                         start=(k == 0), stop=(k == KC - 1))
                    sink(m, pt)

            def sink_a2a(idx):
                def s(m, pt):
                    st = sc.tile([128, TB], BF16, tag="g2")
                    nc.vector.tensor_copy(st[:], pt[:])
                    buf = a2aA_in if m % 2 == 0 else a2aB_in
                    nc.sync.dma_start(buf[m // 2, idx], st[:])
                return s

            # ---- v projection first (its A2A covers the r/k/w evens) ----
            xv_t = make_mix(2, "mA")
            for cc in range(4):
                pvs = [pp(128, TB) for _ in range(4)]
                for k in range(KC):
                    wv_t = wstr.tile([128, TB], BF16, tag="wv_s", bufs=2)
                    nc.sync.dma_start(
                        wv_t[:], Wv[128 * k:128 * (k + 1),
                                    512 * cc:512 * (cc + 1)])
                    for t4 in range(4):
                        nc.tensor.matmul(
                            pvs[t4][:], xv_t[:, k, 128 * t4:128 * (t4 + 1)],
                            wv_t[:], start=(k == 0), stop=(k == KC - 1))
                for t4 in range(4):
                    st = sc.tile([128, TB], BF16, tag="g2")
                    nc.vector.tensor_copy(st[:], pvs[t4][:])
                    for half in range(2):
                        nc.sync.dma_start(
                            a2v_in[2 * cc + half, 128 * t4:128 * (t4 + 1), :],
                            st[:, 256 * half:256 * (half + 1)])

            nc.gpsimd.collective_compute(
                "AllToAll", ALU.bypass, replica_groups=RG,
                ins=[a2v_in[:]], outs=[a2v_out[:]])

            # ---- r/k/w projections, even channel-halves then odd ----
            EV = list(range(0, KC, 2))
            OD = list(range(1, KC, 2))
            xr_t = make_mix(3, "mA")
            xk_t = make_mix(1, "mB")
            # w-decay mix is transient: consumed chunk-by-chunk into t1p
            t1p = pp(64, TB)
            for k in range(KC):
                w2s = wstr.tile([32, 128], BF16, tag="w2s")
                nc.sync.dma_start(w2s[:], maa_w2p[:, 0, k, :])
                mp = pp(128, TB)
                nc.tensor.matmul(mp[:], w2s[:], aTs[0][:],
                                 start=True, stop=True)
                xwk = sc.tile([128, TB], BF16, tag="xxx")
                nc.vector.scalar_tensor_tensor(
                    xwk[:], mp[:], c_tm[:, k, 1:2], xx[:, k, :],
                    ALU.add, ALU.mult)
                nc.vector.tensor_add(xwk[:], xwk[:], ht[:, k, 1:TB + 1])
                nc.tensor.matmul(t1p[:], c_td1[:, k, :], xwk[:],
                                 start=(k == 0), stop=(k == KC - 1))
            t1 = cpool.tile([64, TB], BF16, tag="t1")
            nc.scalar.activation(t1[:], t1p[:], AF.Tanh)

            def w_half(ms):
                for m in ms:
                    wp2 = pp(128, TB)
                    nc.tensor.matmul(wp2[:], c_td2[:, m, :], t1[:],
                                     start=True, stop=True)
                    st = sc.tile([128, TB], BF16, tag="g2")
                    nc.vector.tensor_scalar(st[:], wp2[:], c_td[:, m, 0:1],
                                            None, ALU.add)
                    buf = a2aA_in if m % 2 == 0 else a2aB_in
                    nc.sync.dma_start(buf[m // 2, 2], st[:])

            proj_cm(Wp["r"], sink_a2a(0), lambda k: xr_t[:, k, :], EV)
            proj_cm(Wp["k"], sink_a2a(1), lambda k: xk_t[:, k, :], EV)
            w_half(EV)

            nc.gpsimd.collective_compute(
                "AllToAll", ALU.bypass, replica_groups=RG,
                ins=[a2aA_in[:]], outs=[a2aA_out[:]])

            proj_cm(Wp["r"], sink_a2a(0), lambda k: xr_t[:, k, :], OD)
            proj_cm(Wp["k"], sink_a2a(1), lambda k: xk_t[:, k, :], OD)
            w_half(OD)

            nc.gpsimd.collective_compute(
                "AllToAll", ALU.bypass, replica_groups=RG,
                ins=[a2aB_in[:]], outs=[a2aB_out[:]])

            # ---- g projection (overlaps the odd-half collective) ----
            xg_t = make_mix(4, "mA")

            def sink_g(m, pt):
                nc.scalar.activation(gsb[:, m, :], pt[:], AF.Silu)
            proj_cm(Wp["g"], sink_g, lambda k: xg_t[:, k, :], list(range(KC)))

            # ============ WKV (chunked linear attention) ============
            # loads that wait on collective outputs go through nc.scalar
            # (ACT HWDGE) so they can't head-of-line-block the SP queue.
            for hp in range(2):
                srcRKW = a2aA_out if hp == 0 else a2aB_out
                dstY = a2b0_in if hp == 0 else a2b1_in
                S2s = {}
                for b in range(2):
                    S2s[b] = cpool.tile([128, 64], BF16, tag=f"S_{hp}_{b}",
                                        name="S2t")
                    nc.vector.memset(S2s[b][:], 0.0)
                for jb in range(4):
                    for b in range(2):
                        j = 4 * b + jb
                        S2 = S2s[b]
                        hs = slice(128 * hp, 128 * (hp + 1))
                        rkw = scw.tile([128, 3, TB], BF16, tag="wkv_rkw",
                                       bufs=2)
                        v2 = scw.tile([128, 4, 128], BF16, tag="wkv_v", bufs=2)
                        nc.scalar.dma_start(
                            rkw[:], srcRKW[j].rearrange("c p t -> p c t"))
                        nc.scalar.dma_start(
                            v2[:], a2v_out[j, :, hs]
                            .rearrange("(cc p) c -> p cc c", p=128))
                        r2, k2, w2 = rkw[:, 0, :], rkw[:, 1, :], rkw[:, 2, :]
                        e = scw.tile([128, TB], BF16, tag="wkv_e", bufs=2)
                        nc.scalar.activation(e[:], w2, AF.Exp)
                        qe = scw.tile([128, TB], BF16, tag="wkv_qe", bufs=2)
                        for cc in range(4):
                            cs = slice(128 * cc, 128 * (cc + 1))
                            nc.vector.tensor_tensor_scan(
                                qe[:, cs], e[:, cs], e[:, cs], 0.0,
                                ALU.add, ALU.bypass)
                        ku = scw.tile([128, TB], BF16, tag="wkv_ku", bufs=2)
                        nc.vector.tensor_scalar(ku[:], k2,
                                                c_u[:, hp:hp + 1], None,
                                                ALU.mult)
                        e2f = scw.tile([128, TB], BF16, tag="wkv_e2f", bufs=2)
                        nc.vector.tensor_mul(e2f[:], r2, ku[:])
                        # rt = r*exp(e-qe), kt = k*exp(qe)  (bf16)
                        nc.vector.tensor_sub(e[:], e[:], qe[:])
                        eb = scw.tile([128, TB], BF16, tag="wkv_eb", bufs=2)
                        nc.scalar.activation(eb[:], e[:], AF.Exp)
                        rt = scw.tile([128, TB], BF16, tag="wkv_rt", bufs=2)
                        nc.vector.tensor_mul(rt[:], r2, eb[:])
                        ktb = scw.tile([128, TB], BF16, tag="wkv_eb", bufs=2,
                                       name="ktb")
                        nc.scalar.activation(ktb[:], qe[:], AF.Exp)
                        kt = scw.tile([128, TB], BF16, tag="wkv_kt", bufs=2)
                        nc.vector.tensor_mul(kt[:], k2, ktb[:])
                        ypb = sc.tile([128, TB], BF16, tag="wkv_ypb",
                                      bufs=2, name="ypb")
                        for cc in range(4):
                            cs = slice(128 * cc, 128 * (cc + 1))
                            qend = qe[:, 128 * cc + 127:128 * cc + 128]
                            pl2 = sc.tile([128, 1], F32, tag="wkv_pl")
                            nc.scalar.activation(pl2[:], qend, AF.Exp,
                                                 scale=-1.0)
                            kh = sc.tile([128, 128], BF16, tag="wkv_kh")
                            nc.vector.tensor_scalar(kh[:], kt[:, cs], pl2[:],
                                                    None, ALU.mult)
                            khT = ppb(128, 128)
                            nc.tensor.transpose(khT[:], kh[:], c_id[:])
                            khTs = sc.tile([128, 128], BF16, tag="wkv_khTs")
                            nc.vector.tensor_copy(khTs[:], khT[:])
                            ypk = sc.tile([128, 128], BF16, tag="wkv_ypk")
                            for hh in range(2):
                                h64 = slice(64 * hh, 64 * (hh + 1))
                                at = pp(128, 128)
                                nc.tensor.matmul(at[:], kt[h64, cs],
                                                 rt[h64, cs],
                                                 start=True, stop=True)
                                scol = pp(128, 1)
                                nc.tensor.matmul(scol[:], e2f[h64, cs],
                                                 ones_col[h64, :],
                                                 start=True, stop=True)
                                am = sc.tile([128, 128], BF16, tag="wkv_am")
                                nc.vector.tensor_mul(am[:], at[:], c_msk[:])
                                nc.vector.scalar_tensor_tensor(
                                    am[:], c_id[:], scol[:], am[:],
                                    ALU.mult, ALU.add)
                                yp = pp(128, 64)
                                nc.tensor.matmul(yp[:], am[:], v2[:, cc, h64],
                                                 start=True, stop=False)
                                nc.tensor.matmul(yp[:], rt[h64, cs],
                                                 S2[h64, :],
                                                 start=False, stop=True)
                                sps = pp(64, 64)
                                nc.tensor.matmul(sps[:], khTs[:, h64],
                                                 v2[:, cc, h64],
                                                 start=True, stop=True)
                                nc.vector.scalar_tensor_tensor(
                                    S2[h64, :], S2[h64, :], pl2[h64, :],
                                    sps[:], ALU.mult, ALU.add)
                                nc.vector.tensor_copy(ypk[:, h64], yp[:])
                            ypT = ppb(128, 128)
                            nc.tensor.transpose(ypT[:], ypk[:], c_id[:])
                            nc.vector.tensor_copy(ypb[:, cs], ypT[:])
                        nc.sync.dma_start(dstY[j], ypb[:])
                if hp == 0:
                    nc.gpsimd.collective_compute(
                        "AllToAll", ALU.bypass, replica_groups=RG,
                        ins=[a2b0_in[:]], outs=[a2b0_out[:]])
            nc.gpsimd.collective_compute(
                "AllToAll", ALU.bypass, replica_groups=RG,
                ins=[a2b1_in[:]], outs=[a2b1_out[:]])

            # ============ y assembly + GroupNorm (token domain) ============
            yt = big.tile([128, KC, TB], BF16, tag="mA", name="yt")
            psS, psSq = pp(32, TB), pp(32, TB)
            for m in range(KC):
                src = a2b0_out if m % 2 == 0 else a2b1_out
                nc.scalar.dma_start(yt[:, m, :], src[m // 2])
                sq = sc.tile([128, TB], BF16, tag="g1")
                nc.scalar.activation(sq[:], yt[:, m, :], AF.Square)
                st, sp = (m == 0), (m == KC - 1)
                nc.tensor.matmul(psS[:], c_gsel[:, m, :], yt[:, m, :],
                                 start=st, stop=sp)
                nc.tensor.matmul(psSq[:], c_gsel[:, m, :], sq[:],
                                 start=st, stop=sp)
            gst = lnp.tile([32, 2 * TB], F32, tag="ln_stats")
            gmean, ginv = gst[:, 0:TB], gst[:, TB:]
            nc.scalar.activation(gmean[:], psS[:], AF.Copy, scale=1.0 / 64)
            nc.scalar.activation(ginv[:], psSq[:], AF.Copy, scale=1.0 / 64)
            gvar = lnp.tile([32, TB], F32, tag="ln_work")
            nc.vector.tensor_mul(gvar[:], gmean[:], gmean[:])
            nc.vector.tensor_sub(gvar[:], ginv[:], gvar[:])
            nc.scalar.activation(ginv[:], gvar[:], AF.Sqrt, bias=EPS_LNX)
            nc.vector.reciprocal(ginv[:], ginv[:])
            gstb = lnp.tile([32, 2 * TB], BF16, tag="ln_stb")
            nc.vector.tensor_copy(gstb[:], gst[:])
            for m in range(KC):
                bcM = pp(128, TB)
                nc.tensor.matmul(bcM[:], c_bsel[:, m, :], gstb[:, 0:TB],
                                 start=True, stop=True)
                bcI = pp(128, TB)
                nc.tensor.matmul(bcI[:], c_bsel[:, m, :], gstb[:, TB:],
                                 start=True, stop=True)
                yn = sc.tile([128, TB], BF16, tag="g1")
                nc.vector.tensor_sub(yn[:], yt[:, m, :], bcM[:])
                nc.vector.tensor_mul(yn[:], yn[:], bcI[:])
                nc.vector.tensor_scalar(yn[:], yn[:], c_lnx[:, m, 0:1],
                                        c_lnx[:, m, 1:2], ALU.mult, ALU.add)
                nc.vector.tensor_mul(yt[:, m, :], yn[:], gsb[:, m, :])

            # ============ att out proj + residual + LN2 stats ============
            x2b = big.tile([128, KC, TB], BF16, tag="xb", name="x2b")
            psA3, psA4 = pp(1, TB), pp(1, TB)
            for m in range(KC):
                wt = wstr.tile([128, KC, 128], BF16, tag="wstream", bufs=2)
                nc.sync.dma_start(wt[:], Wp["o"][m])
                pt = pp(128, TB)
                for k in range(KC):
                    nc.tensor.matmul(pt[:], wt[:, k, :], yt[:, k, :],
                                     start=(k == 0), stop=(k == KC - 1))
                x2t = sc.tile([128, TB], F32, tag="g3", bufs=2)
                xin = sc.tile([128, TB], F32, tag="g4", bufs=2)
                nc.sync.dma_start(xin[:], xT[128 * m:128 * (m + 1), 1:TB + 1])
                nc.vector.tensor_add(x2t[:], pt[:], xin[:])
                nc.sync.dma_start(x2d[128 * m:128 * (m + 1), :], x2t[:])
                nc.scalar.activation(x2b[:, m, :], x2t[:], AF.Copy)
                sq = sc.tile([128, TB], BF16, tag="g1")
                nc.scalar.activation(sq[:], x2b[:, m, :], AF.Square)
                st, sp = (m == 0), (m == KC - 1)
                nc.tensor.matmul(psA3[:], ones_col[:], x2b[:, m, :],
                                 start=st, stop=sp)
                nc.tensor.matmul(psA4[:], ones_col[:], sq[:],
                                 start=st, stop=sp)

            # ---- ln2 normalize ----
            stats2 = lnp.tile([1, 2 * TB], F32, tag="ln_stats")
            mean2, msq2 = stats2[:, 0:TB], stats2[:, TB:]
            nc.scalar.activation(mean2[:], psA3[:], AF.Copy, scale=1.0 / C)
            nc.scalar.activation(msq2[:], psA4[:], AF.Copy, scale=1.0 / C)
            wk2 = lnp.tile([1, TB], F32, tag="ln_work")
            nc.vector.tensor_mul(wk2[:], mean2[:], mean2[:])
            nc.vector.tensor_sub(wk2[:], msq2[:], wk2[:])
            nc.scalar.activation(wk2[:], wk2[:], AF.Sqrt, bias=EPS_LN)
            nc.vector.reciprocal(wk2[:], wk2[:])
            stb2 = lnp.tile([1, 2 * TB], BF16, tag="ln_stb")
            nc.vector.tensor_copy(stb2[:, 0:TB], mean2[:])
            nc.vector.tensor_copy(stb2[:, TB:], wk2[:])
            bmp3, bip3 = pp(128, TB), pp(128, TB)
            nc.tensor.matmul(bmp3[:], ones_row[:], stb2[:, 0:TB], start=True,
                             stop=True)
            nc.tensor.matmul(bip3[:], ones_row[:], stb2[:, TB:], start=True,
                             stop=True)
            bc2 = lnp.tile([128, 2 * TB], BF16, tag="ln_bc")
            nc.vector.tensor_copy(bc2[:, 0:TB], bmp3[:])
            nc.vector.tensor_copy(bc2[:, TB:], bip3[:])
            for k in range(KC):
                t = sc.tile([128, TB], BF16, tag="e2")
                nc.vector.tensor_sub(t[:], x2b[:, k, :], bc2[:, 0:TB])
                nc.vector.tensor_mul(t[:], t[:], bc2[:, TB:])
                nc.vector.tensor_scalar(ht[:, k, 1:TB + 1], t[:],
                                        c_ln2[:, k, 0:1], c_ln2[:, k, 1:2],
                                        ALU.mult, ALU.add)
                # h2 boundary (own last token) -> ag_in for the neighbor
                nc.sync.dma_start(ag_in[0:1, 128 * k:128 * (k + 1)],
                                  ht[:, k, TB:TB + 1])

            nc.gpsimd.collective_compute(
                "AllGather", ALU.bypass, replica_groups=RG,
                ins=[ag_in[:]], outs=[ag_out[:]])

            for q in range(4):
                agp = sc.tile([NCORE, TB], BF16, tag="agp", bufs=1)
                nc.scalar.dma_start(agp[:], ag_out[:, 512 * q:512 * (q + 1)])
                hp_ = pp(1, TB)
                nc.tensor.matmul(hp_[:], c_sel[:], agp[:],
                                 start=True, stop=True)
                hrow = sc.tile([1, TB], BF16, tag="hrow")
                nc.vector.tensor_copy(hrow[:], hp_[:])
                for mm in range(4):
                    m = 4 * q + mm
                    nc.sync.dma_start(ht[:, m, 0:1],
                                      hrow[0:1, 128 * mm:128 * (mm + 1)])

            for k in range(KC):
                xx2 = sc.tile([128, TB], BF16, tag="g1")
                nc.vector.tensor_sub(xx2[:], ht[:, k, 0:TB],
                                     ht[:, k, 1:TB + 1])
                nc.vector.scalar_tensor_tensor(
                    xx[:, k, :], xx2[:], c_cm[:, k, 0:1], ht[:, k, 1:TB + 1],
                    ALU.mult, ALU.add)        # xk2
                nc.vector.scalar_tensor_tensor(
                    gsb[:, k, :], xx2[:], c_cm[:, k, 1:2], ht[:, k, 1:TB + 1],
                    ALU.mult, ALU.add)        # xr2

            # ============ FFN ============
            kfA = big.tile([128, KC, TB], BF16, tag="mB")
            kfB = big.tile([128, KC, TB], BF16, tag="mA")
            kfC = big.tile([128, KC, TB], BF16, tag="ht")
            kfD = big.tile([128, 8, TB], BF16, tag="xb")

            def kf_view(i):
                if i < KC:
                    return kfA[:, i, :]
                if i < 32:
                    return kfB[:, i - 16, :]
                return kfC[:, i - 32, :] if i < 48 else kfD[:, i - 48, :]

            for mf in range(KF):
                wt = wstr.tile([128, KC, 128], BF16, tag="wstream", bufs=2)
                nc.sync.dma_start(wt[:], Wck_p[mf])
                pt = pp(128, TB)
                for k in range(KC):
                    nc.tensor.matmul(pt[:], wt[:, k, :], xx[:, k, :],
                                     start=(k == 0), stop=(k == KC - 1))
                rl = sc.tile([128, TB], BF16, tag="g1")
                nc.vector.tensor_scalar(rl[:], pt[:], 0.0, None, ALU.max)
                nc.scalar.activation(kf_view(mf), rl[:], AF.Square)

            for m in range(KC):
                ptu = pp(128, TB)
                for q in range(4):
                    wcv = wstr.tile([128, 14, 128], BF16, tag="wcv_s", bufs=2)
                    nc.sync.dma_start(wcv[:],
                                      Wcv_p[m, :, q * 14:(q + 1) * 14, :])
                    for kk in range(14):
                        ki = q * 14 + kk
                        nc.tensor.matmul(ptu[:], wcv[:, kk, :], kf_view(ki),
                                         start=(ki == 0), stop=(ki == KF - 1))
                wt = wstr.tile([128, KC, 128], BF16, tag="wstream", bufs=2)
                nc.sync.dma_start(wt[:], Wp["cr"][m])
                pts = pp(128, TB)
                for k in range(KC):
                    nc.tensor.matmul(pts[:], wt[:, k, :], gsb[:, k, :],
                                     start=(k == 0), stop=(k == KC - 1))
                ssb = sc.tile([128, TB], BF16, tag="g2")
                nc.scalar.activation(ssb[:], pts[:], AF.Sigmoid)
                ot = sc.tile([128, TB], F32, tag="g3", bufs=2)
                x2in = sc.tile([128, TB], F32, tag="g4", bufs=2)
                nc.sync.dma_start(x2in[:],
                                  x2d[128 * m:128 * (m + 1), :])
                nc.vector.tensor_mul(ot[:], ptu[:], ssb[:])
                nc.vector.tensor_add(ot[:], ot[:], x2in[:])
                nc.sync.dma_start(outT[128 * m:128 * (m + 1), :], ot[:])

    nc.compile()
    return nc


_CACHE = {}


def _get_program():
    if "nc" not in _CACHE:
        _CACHE["nc"] = build_program()
    return _CACHE["nc"]


def _pret4(w):
    """(Cin, Cout) -> (Cout/128, 128, Cin/128, 128): [m,p,k,f] = w[128k+p, 128m+f]
    so each m-tile is one fully-contiguous per-partition DMA."""
    ci, co = w.shape
    return np.ascontiguousarray(
        w.reshape(ci // 128, 128, co // 128, 128).transpose(2, 1, 0, 3))


def _shard_inputs(inp):
    f32 = np.float32
    x = np.asarray(inp["x"], f32)
    bf = lambda a: np.asarray(a, f32).astype(NP_BF16)

    maa_w2 = np.asarray(inp["maa_w2"], f32)
    w2p = np.zeros((32, 5, KC, 128), f32)
    for i in range(5):
        for m in range(KC):
            w2p[:, i, m, :] = maa_w2[i][:, 128 * m:128 * (m + 1)]
    td_w2 = np.asarray(inp["td_w2"], f32)
    td2p = td_w2.reshape(64, KC, 128).copy()
    for m in range(KC):
        td2p[:, m, :] = td_w2[:, 128 * m:128 * (m + 1)]

    gsel = np.zeros((128, KC, 32), f32)
    bsel = np.zeros((32, KC, 128), f32)
    for p in range(128):
        for k in range(KC):
            gsel[p, k, 2 * k + p // 64] = 1.0
            bsel[2 * k + p // 64, k, p] = 1.0

    shared = {
        "ln1_wb": np.stack([inp["ln1_w"], inp["ln1_b"]], 1).astype(f32),
        "ln2_wb": np.stack([inp["ln2_w"], inp["ln2_b"]], 1).astype(f32),
        "lnx_wb": np.stack([inp["lnx_w"], inp["lnx_b"]], 1).astype(f32),
        "tm_maaT": np.asarray(inp["tm_maa"], f32).T.copy(),
        "cm_maaT": np.asarray(inp["cm_maa"], f32).T.copy(),
        "td_col": np.asarray(inp["time_decay"], f32).reshape(C, 1),
        "ident": np.eye(128, dtype=f32).astype(NP_BF16),
        "mask_su": np.triu(np.ones((128, 128), f32), 1).astype(NP_BF16),
        "gsel": gsel.reshape(128, KC * 32).astype(NP_BF16),
        "bsel": bsel.reshape(32, KC * 128).astype(NP_BF16),
        "maa_w1": bf(inp["maa_w1"]),
        "maa_w2p": w2p.astype(NP_BF16),
        "td_w1": bf(inp["td_w1"]),
        "td_w2p": td2p.astype(NP_BF16),
        "Wr_p": bf(_pret4(np.asarray(inp["Wr"], f32))),
        "Wk_p": bf(_pret4(np.asarray(inp["Wk"], f32))),
        "Wg_p": bf(_pret4(np.asarray(inp["Wg"], f32))),
        "Wo_p": bf(_pret4(np.asarray(inp["Wo"], f32))),
        "Wcr_p": bf(_pret4(np.asarray(inp["Wcr"], f32))),
        "Wv": bf(inp["Wv"]),
        "Wck_p": bf(_pret4(np.asarray(inp["Wck"], f32))),
        "Wcv_p": bf(_pret4(np.asarray(inp["Wcv"], f32))),
    }
    u = np.asarray(inp["time_faaaa"], f32).reshape(C)

    in_maps = []
    for c in range(NCORE):
        b, blk = c // 4, c % 4
        ts = blk * TB
        xe = np.zeros((C, TB + 1), f32)
        xe[:, 1:] = x[b, ts:ts + TB].T
        if blk > 0:
            xe[:, 0] = x[b, ts - 1]
        ul = u[LCH * c:LCH * (c + 1)].reshape(2, 128).T.copy()
        sel = np.zeros((NCORE, 1), NP_BF16)
        if blk > 0:
            sel[c - 1, 0] = 1.0
        m = dict(shared)
        m.update({
            "xT": xe,
            "halo_mask": np.full((128, 1), 1.0 if blk > 0 else 0.0, f32),
            "sel_prev": sel,
            "u_loc": ul,
        })
        in_maps.append(m)
    return in_maps


def run(inputs, trace=False):
    nc = _get_program()
    in_maps = _shard_inputs(inputs)
    res = bass_utils.run_bass_kernel_spmd(
        nc, in_maps, core_ids=list(range(NCORE)), trace=trace)
    x = np.asarray(inputs["x"], np.float32)
    out = np.empty_like(x)
    for c in range(NCORE):
        b, blk = c // 4, c % 4
        out[b, blk * TB:(blk + 1) * TB, :] = np.asarray(
            res.results[c]["out"], np.float32).T
    return out, res.exec_time_ns


def kernel(**inputs):
    out, _ = run(inputs)
    return out


if __name__ == "__main__":
    build_program()
    print("build ok")


# revision 17
# speedup vs baseline: 1.0149x; 1.0149x over previous
"""RWKV6 block (nn_Block_14602888806424) on 8 Trainium2 NeuronCores.

Token-sharded (sequence-parallel): each core owns 512 tokens (B=2 x 4
blocks); matmuls/LNs/mixing are token-local in channel-major layout.
r/k/w/v are redistributed head-sharded via AllToAll around the chunked
(L=128) WKV linear-attention scan (4 heads/core); a second pair of
AllToAlls (split by head-pair half) returns raw y; GroupNorm is applied
in the token-sharded domain with PE-based group stats. A small AllGather
carries the 1-token boundary halo for the second token-shift.

Perf structure: all collective payloads are bf16; the forward exchange is
split (v first, then r/k/w even channel-halves, then odd) so projection
matmuls overlap collective time; DMA loads that wait on collective
results issue from the ACT HWDGE queue so they cannot head-of-line-block
the SP DMA queue; weights are host-pre-tiled so every streamed weight
tile is a contiguous per-partition DMA; matmul operands are bf16 (fp32
matmuls run at 1/4 rate).
"""

import sys
import numpy as np

sys.path.insert(0, "/opt/trn_rl_repo")

import concourse.bass as bass
import concourse.bacc as bacc
import concourse.mybir as mybir
import concourse.tile as tile
from concourse.tile_rust import add_dep_helper
from concourse import bass_utils

F32 = mybir.dt.float32
BF16 = mybir.dt.bfloat16
NP_BF16 = mybir.dt.np(BF16)
AF = mybir.ActivationFunctionType
ALU = mybir.AluOpType

B, T, C, H, N, FF = 2, 2048, 2048, 32, 64, 7168
D_MIX, D_DECAY = 32, 64
EPS_LN = 1e-5
EPS_LNX = 1e-5 * 8.0**2
NCORE = 8
TB = 512
KC = C // 128          # 16
KF = FF // 128         # 56
LCH = 256              # channels per core (4 heads)
RG = [list(range(NCORE))]


def build_program():
    nc = bacc.Bacc("TRN2", target_bir_lowering=False, debug=False,
                   num_devices=NCORE, enable_asserts=False)

    def din(name, shape, dt=F32):
        return nc.dram_tensor(name, list(shape), dt, kind="ExternalInput").ap()

    xT = din("xT", (C, TB + 1))
    halo_mask = din("halo_mask", (128, 1))
    sel_prev = din("sel_prev", (NCORE, 1), BF16)
    u_loc = din("u_loc", (128, 2))
    lnx_wb = din("lnx_wb", (C, 2))
    ln1_wb = din("ln1_wb", (C, 2))
    ln2_wb = din("ln2_wb", (C, 2))
    tm_maaT = din("tm_maaT", (C, 6))
    cm_maaT = din("cm_maaT", (C, 2))
    td_col = din("td_col", (C, 1))
    ident = din("ident", (128, 128), BF16)
    mask_su = din("mask_su", (128, 128), BF16)
    gsel = din("gsel", (128, KC * 32), BF16)
    bsel = din("bsel", (32, KC * 128), BF16)
    maa_w1 = din("maa_w1", (C, 5 * D_MIX), BF16)
    maa_w2p = din("maa_w2p", (32, 5, KC, 128), BF16)
    td_w1 = din("td_w1", (C, D_DECAY), BF16)
    td_w2p = din("td_w2p", (64, KC, 128), BF16)
    Wp = {k: din(f"W{k}_p", (KC, 128, KC, 128), BF16)
          for k in ["r", "k", "g", "o", "cr"]}
    Wv = din("Wv", (C, C), BF16)
    Wck_p = din("Wck_p", (KF, 128, KC, 128), BF16)
    Wcv_p = din("Wcv_p", (KC, 128, KF, 128), BF16)

    outT = nc.dram_tensor("out", [C, TB], F32, kind="ExternalOutput").ap()

    with tile.TileContext(nc) as tc:
        import contextlib
        with contextlib.ExitStack() as ctx:
            dram = ctx.enter_context(tc.tile_pool(name="dram", bufs=1,
                                                  space="DRAM"))
            cpool = ctx.enter_context(tc.tile_pool(name="const", bufs=1))
            big = ctx.enter_context(tc.tile_pool(name="big", bufs=1))
            wstr = ctx.enter_context(tc.tile_pool(name="wstr", bufs=3))
            sc = ctx.enter_context(tc.tile_pool(name="scratch", bufs=2))
            scw = ctx.enter_context(tc.tile_pool(name="scw", bufs=1))
            lnp = ctx.enter_context(tc.tile_pool(name="lnp", bufs=1))
            ps = ctx.enter_context(
                tc.tile_pool(name="psum", bufs=8, space="PSUM"))

            def pp(p_, f_):
                return ps.tile([p_, f_], F32, tag="pp", name="pp")

            def ppb(p_, f_):
                return ps.tile([p_, f_], BF16, tag="pp", name="ppb")

            # ---- DRAM internals (all collective payloads bf16) ----
            a2aA_in = dram.tile([NCORE, 3, 128, TB], BF16, tag="a2aA_in")
            a2aA_out = dram.tile([NCORE, 3, 128, TB], BF16, tag="a2aA_out")
            a2aB_in = dram.tile([NCORE, 3, 128, TB], BF16, tag="a2aB_in")
            a2aB_out = dram.tile([NCORE, 3, 128, TB], BF16, tag="a2aB_out")
            a2v_in = dram.tile([NCORE, TB, LCH], BF16, tag="a2v_in")
            a2v_out = dram.tile([NCORE, TB, LCH], BF16, tag="a2v_out")
            a2b0_in = dram.tile([NCORE, 128, TB], BF16, tag="a2b0_in")
            a2b0_out = dram.tile([NCORE, 128, TB], BF16, tag="a2b0_out")
            a2b1_in = dram.tile([NCORE, 128, TB], BF16, tag="a2b1_in")
            a2b1_out = dram.tile([NCORE, 128, TB], BF16, tag="a2b1_out")
            ag_in = dram.tile([1, C], BF16, tag="ag_in")
            ag_out = dram.tile([NCORE, C], BF16, tag="ag_out",
                               addr_space="Shared")
            x2d = dram.tile([C, TB], F32, tag="x2d")

            # ---- constants ----
            def cload(name, src, shape, dt=F32, rearr=None):
                t = cpool.tile(list(shape), dt, tag=name)
                nc.sync.dma_start(t[:], src if rearr is None
                                  else src.rearrange(rearr, p=128))
                return t

            c_ln1 = cload("c_ln1", ln1_wb, (128, KC, 2), F32, "(k p) f -> p k f")
            c_ln2 = cload("c_ln2", ln2_wb, (128, KC, 2), F32, "(k p) f -> p k f")
            c_lnx = cload("c_lnx", lnx_wb, (128, KC, 2), F32, "(k p) f -> p k f")
            c_tm = cload("c_tm", tm_maaT, (128, KC, 6), F32, "(k p) f -> p k f")
            c_cm = cload("c_cm", cm_maaT, (128, KC, 2), F32, "(k p) f -> p k f")
            c_td = cload("c_td", td_col, (128, KC, 1), F32, "(k p) f -> p k f")
            c_hm = cload("c_hm", halo_mask, (128, 1))
            c_sel = cload("c_sel", sel_prev, (NCORE, 1), BF16)
            c_u = cload("c_u", u_loc, (128, 2))
            c_id = cload("c_id", ident, (128, 128), BF16)
            c_msk = cload("c_msk", mask_su, (128, 128), BF16)
            c_gsel = cload("c_gsel", gsel, (128, KC, 32), BF16)
            c_bsel = cload("c_bsel", bsel, (32, KC, 128), BF16)
            c_w1 = cload("c_w1", maa_w1, (128, KC, 5 * D_MIX), BF16,
                         "(k p) f -> p k f")
            c_td1 = cload("c_td1", td_w1, (128, KC, D_DECAY), BF16,
                          "(k p) f -> p k f")
            c_td2 = cload("c_td2", td_w2p, (64, KC, 128), BF16)
            ones_col = cpool.tile([128, 1], BF16, tag="ones_col")
            nc.vector.memset(ones_col[:], 1.0)
            ones_row = cpool.tile([1, 128], BF16, tag="ones_row")
            nc.vector.memset(ones_row[:], 1.0)
            for _cv in (EPS_LN, EPS_LNX):
                cvt = cpool.tile([128, 1], F32, tag=f"cv{_cv}", name="cvt")
                nc.vector.memset(cvt[:], _cv)
                nc.const_aps.aps[(F32, _cv)] = cvt[:]

            # ---- persistent SBUF ----
            xb = big.tile([128, KC, TB + 1], BF16, tag="xb")
            ht = big.tile([128, KC, TB + 1], BF16, tag="ht")
            xx = big.tile([128, KC, TB], BF16, tag="xx")      # later xk2
            gsb = big.tile([128, KC, TB], BF16, tag="gsb")    # later xr2

            # ============ LN1 over TB+1 cols (src resident in xb) ============
            psA, psB = pp(1, TB), pp(1, 1)
            psA2, psB2 = pp(1, TB), pp(1, 1)
            for k in range(KC):
                t = sc.tile([128, TB + 1], F32, tag="e1", bufs=2)
                nc.sync.dma_start(t[:], xT[128 * k:128 * (k + 1), :])
                nc.scalar.activation(xb[:, k, :], t[:], AF.Copy)
                sq = sc.tile([128, TB + 1], BF16, tag="e2")
                nc.scalar.activation(sq[:], xb[:, k, :], AF.Square)
                st, sp = (k == 0), (k == KC - 1)
                nc.tensor.matmul(psA[:], ones_col[:], xb[:, k, 0:TB],
                                 start=st, stop=sp)
                nc.tensor.matmul(psB[:], ones_col[:], xb[:, k, TB:TB + 1],
                                 start=st, stop=sp)
                nc.tensor.matmul(psA2[:], ones_col[:], sq[:, 0:TB],
                                 start=st, stop=sp)
                nc.tensor.matmul(psB2[:], ones_col[:], sq[:, TB:TB + 1],
                                 start=st, stop=sp)
            stats = lnp.tile([1, 2 * (TB + 1)], F32, tag="ln_stats")
            mean, msq = stats[:, 0:TB + 1], stats[:, TB + 1:]
            nc.scalar.activation(mean[:, 0:TB], psA[:], AF.Copy, scale=1.0 / C)
            nc.scalar.activation(mean[:, TB:TB + 1], psB[:], AF.Copy,
                                 scale=1.0 / C)
            nc.scalar.activation(msq[:, 0:TB], psA2[:], AF.Copy, scale=1.0 / C)
            nc.scalar.activation(msq[:, TB:TB + 1], psB2[:], AF.Copy,
                                 scale=1.0 / C)
            wk = lnp.tile([1, TB + 1], F32, tag="ln_work")
            nc.vector.tensor_mul(wk[:], mean[:], mean[:])
            nc.vector.tensor_sub(wk[:], msq[:], wk[:])
            nc.scalar.activation(wk[:], wk[:], AF.Sqrt, bias=EPS_LN)
            nc.vector.reciprocal(wk[:], wk[:])
            stb = lnp.tile([1, 2 * (TB + 1)], BF16, tag="ln_stb")
            nc.vector.tensor_copy(stb[:, 0:TB + 1], mean[:])
            nc.vector.tensor_copy(stb[:, TB + 1:], wk[:])
            bmp, bmp2 = pp(128, TB), pp(128, 1)
            bip, bip2 = pp(128, TB), pp(128, 1)
            nc.tensor.matmul(bmp[:], ones_row[:], stb[:, 0:TB],
                             start=True, stop=True)
            nc.tensor.matmul(bmp2[:], ones_row[:], stb[:, TB:TB + 1],
                             start=True, stop=True)
            nc.tensor.matmul(bip[:], ones_row[:], stb[:, TB + 1:2 * TB + 1],
                             start=True, stop=True)
            nc.tensor.matmul(bip2[:], ones_row[:], stb[:, 2 * TB + 1:],
                             start=True, stop=True)
            bc = lnp.tile([128, 2 * (TB + 1)], BF16, tag="ln_bc")
            bm, bi = bc[:, 0:TB + 1], bc[:, TB + 1:]
            nc.vector.tensor_copy(bm[:, 0:TB], bmp[:])
            nc.vector.tensor_copy(bm[:, TB:TB + 1], bmp2[:])
            nc.vector.tensor_copy(bi[:, 0:TB], bip[:])
            nc.vector.tensor_copy(bi[:, TB:TB + 1], bip2[:])
            for k in range(KC):
                tn = sc.tile([128, TB + 1], BF16, tag="e2")
                nc.vector.tensor_sub(tn[:], xb[:, k, :], bm[:])
                nc.vector.tensor_mul(tn[:], tn[:], bi[:])
                d = ht[:, k, :]
                nc.vector.tensor_scalar(d, tn[:], c_ln1[:, k, 0:1],
                                        c_ln1[:, k, 1:2], ALU.mult, ALU.add)
                nc.vector.tensor_scalar(d[:, 0:1], d[:, 0:1], c_hm[:],
                                        None, ALU.mult)
                nc.vector.tensor_sub(xx[:, k, :], ht[:, k, 0:TB],
                                     ht[:, k, 1:TB + 1])

            # ============ maa ============
            aps1, aps2 = pp(128, TB), pp(32, TB)
            for k in range(KC):
                xxx = sc.tile([128, TB], BF16, tag="xxx")
                nc.vector.scalar_tensor_tensor(
                    xxx[:], xx[:, k, :], c_tm[:, k, 0:1], ht[:, k, 1:TB + 1],
                    ALU.mult, ALU.add)
                nc.tensor.matmul(aps1[:], c_w1[:, k, 0:128], xxx[:],
                                 start=(k == 0), stop=(k == KC - 1))
                nc.tensor.matmul(aps2[:], c_w1[:, k, 128:160], xxx[:],
                                 start=(k == 0), stop=(k == KC - 1))
            aTs = [cpool.tile([32, TB], BF16, tag=f"aT{i}", name="aTs")
                   for i in range(5)]
            for i in range(4):
                nc.scalar.activation(aTs[i][:], aps1[32 * i:32 * (i + 1), :],
                                     AF.Tanh)
            nc.scalar.activation(aTs[4][:], aps2[0:32, :], AF.Tanh)

            def make_mix(i, tag):
                mt = big.tile([128, KC, TB], BF16, tag=tag, name="mixbuf")
                for k in range(KC):
                    w2s = wstr.tile([32, 128], BF16, tag="w2s")
                    nc.sync.dma_start(w2s[:], maa_w2p[:, i, k, :])
                    mp = pp(128, TB)
                    nc.tensor.matmul(mp[:], w2s[:], aTs[i][:],
                                     start=True, stop=True)
                    mpc = sc.tile([128, TB], BF16, tag="mpc")
                    nc.scalar.activation(mpc[:], mp[:], AF.Copy)
                    t = sc.tile([128, TB], BF16, tag="g1")
                    nc.vector.scalar_tensor_tensor(
                        t[:], mpc[:], c_tm[:, k, i + 1:i + 2], xx[:, k, :],
                        ALU.add, ALU.mult)
                    nc.vector.tensor_add(mt[:, k, :], t[:],
                                         ht[:, k, 1:TB + 1])
                return mt

            last_dma = {}

            def proj_cm(wp_ap, sink, src_view, ms):
                for m in ms:
                    wt = wstr.tile([128, KC, 128], BF16, tag="wstream", bufs=2)
                    last_dma["wt"] = nc.sync.dma_start(wt[:], wp_ap[m])
                    pt = pp(128, TB)
                    for k in range(KC):
                        nc.tensor.matmul(pt[:], wt[:, k, :], src_view(k),
                                         start=(k == 0), stop=(k == KC - 1))
                    sink(m, pt)

            def sink_a2a(idx):
                def s(m, pt):
                    st = sc.tile([128, TB], BF16, tag="g2")
                    nc.vector.tensor_copy(st[:], pt[:])
                    buf = a2aA_in if m % 2 == 0 else a2aB_in
                    nc.sync.dma_start(buf[m // 2, idx], st[:])
                return s

            # ---- v projection first: its A2A starts before everything ----
            xv_t = make_mix(2, "mA")
            for cc in range(4):
                pvs = [pp(128, TB) for _ in range(4)]
                for k in range(KC):
                    wv_t = wstr.tile([128, TB], BF16, tag="wv_s", bufs=2)
                    nc.sync.dma_start(
                        wv_t[:], Wv[128 * k:128 * (k + 1),
                                    512 * cc:512 * (cc + 1)])
                    for t4 in range(4):
                        nc.tensor.matmul(
                            pvs[t4][:], xv_t[:, k, 128 * t4:128 * (t4 + 1)],
                            wv_t[:], start=(k == 0), stop=(k == KC - 1))
                for t4 in range(4):
                    st = sc.tile([128, TB], BF16, tag="g2")
                    nc.vector.tensor_copy(st[:], pvs[t4][:])
                    for half in range(2):
                        nc.sync.dma_start(
                            a2v_in[2 * cc + half, 128 * t4:128 * (t4 + 1), :],
                            st[:, 256 * half:256 * (half + 1)])

            nc.gpsimd.collective_compute(
                "AllToAll", ALU.bypass, replica_groups=RG,
                ins=[a2v_in[:]], outs=[a2v_out[:]])

            # ---- r/k/w projections, even channel-halves then odd ----
            EV = list(range(0, KC, 2))
            OD = list(range(1, KC, 2))
            xr_t = make_mix(3, "mA")
            xk_t = make_mix(1, "mB")
            # w-decay mix is transient: consumed chunk-by-chunk into t1p
            t1p = pp(64, TB)
            for k in range(KC):
                w2s = wstr.tile([32, 128], BF16, tag="w2s")
                nc.sync.dma_start(w2s[:], maa_w2p[:, 0, k, :])
                mp = pp(128, TB)
                nc.tensor.matmul(mp[:], w2s[:], aTs[0][:],
                                 start=True, stop=True)
                mpc = sc.tile([128, TB], BF16, tag="mpc")
                nc.scalar.activation(mpc[:], mp[:], AF.Copy)
                xwk = sc.tile([128, TB], BF16, tag="xxx")
                nc.vector.scalar_tensor_tensor(
                    xwk[:], mpc[:], c_tm[:, k, 1:2], xx[:, k, :],
                    ALU.add, ALU.mult)
                nc.vector.tensor_add(xwk[:], xwk[:], ht[:, k, 1:TB + 1])
                nc.tensor.matmul(t1p[:], c_td1[:, k, :], xwk[:],
                                 start=(k == 0), stop=(k == KC - 1))
            t1 = cpool.tile([64, TB], BF16, tag="t1")
            nc.scalar.activation(t1[:], t1p[:], AF.Tanh)

            def w_half(ms):
                for m in ms:
                    wp2 = pp(128, TB)
                    nc.tensor.matmul(wp2[:], c_td2[:, m, :], t1[:],
                                     start=True, stop=True)
                    st = sc.tile([128, TB], BF16, tag="g2")
                    nc.vector.tensor_scalar(st[:], wp2[:], c_td[:, m, 0:1],
                                            None, ALU.add)
                    buf = a2aA_in if m % 2 == 0 else a2aB_in
                    nc.sync.dma_start(buf[m // 2, 2], st[:])

            proj_cm(Wp["r"], sink_a2a(0), lambda k: xr_t[:, k, :], EV)
            proj_cm(Wp["k"], sink_a2a(1), lambda k: xk_t[:, k, :], EV)
            w_half(EV)

            nc.gpsimd.collective_compute(
                "AllToAll", ALU.bypass, replica_groups=RG,
                ins=[a2aA_in[:]], outs=[a2aA_out[:]])

            proj_cm(Wp["r"], sink_a2a(0), lambda k: xr_t[:, k, :], OD)
            proj_cm(Wp["k"], sink_a2a(1), lambda k: xk_t[:, k, :], OD)
            w_half(OD)

            nc.gpsimd.collective_compute(
                "AllToAll", ALU.bypass, replica_groups=RG,
                ins=[a2aB_in[:]], outs=[a2aB_out[:]])

            # ---- g projection (overlaps the odd-half collective) ----
            xg_t = make_mix(4, "mB")

            def sink_g(m, pt):
                nc.scalar.activation(gsb[:, m, :], pt[:], AF.Silu)
            proj_cm(Wp["g"], sink_g, lambda k: xg_t[:, k, :], list(range(KC)))

            # ============ WKV (chunked linear attention) ============
            # loads that wait on collective outputs go through nc.scalar
            # (ACT HWDGE) so they can't head-of-line-block the SP queue.
            for hp in range(2):
                srcRKW = a2aA_out if hp == 0 else a2aB_out
                dstY = a2b0_in if hp == 0 else a2b1_in
                S2s = {}
                for b in range(2):
                    S2s[b] = cpool.tile([128, 64], BF16, tag=f"S_{hp}_{b}",
                                        name="S2t")
                    nc.vector.memset(S2s[b][:], 0.0)
                for jb in range(4):
                    for b in range(2):
                        j = 4 * b + jb
                        S2 = S2s[b]
                        hs = slice(128 * hp, 128 * (hp + 1))
                        rkw = scw.tile([128, 3, TB], BF16, tag="wkv_rkw",
                                       bufs=2)
                        v2 = scw.tile([128, 4, 128], BF16, tag="wkv_v", bufs=2)
                        h1 = nc.scalar.dma_start(
                            rkw[:], srcRKW[j].rearrange("c p t -> p c t"))
                        h2 = nc.scalar.dma_start(
                            v2[:], a2v_out[j, :, hs]
                            .rearrange("(cc p) c -> p cc c", p=128))
                        add_dep_helper(h1.ins, last_dma["wt"].ins, sync=False,
                                       reason="dma-lane order")
                        add_dep_helper(h2.ins, last_dma["wt"].ins, sync=False,
                                       reason="dma-lane order")
                        r2, k2, w2 = rkw[:, 0, :], rkw[:, 1, :], rkw[:, 2, :]
                        e = scw.tile([128, TB], BF16, tag="wkv_e", bufs=2)
                        nc.scalar.activation(e[:], w2, AF.Exp)
                        qe = scw.tile([128, TB], BF16, tag="wkv_qe", bufs=2)
                        for cc in range(4):
                            cs = slice(128 * cc, 128 * (cc + 1))
                            nc.vector.tensor_tensor_scan(
                                qe[:, cs], e[:, cs], e[:, cs], 0.0,
                                ALU.add, ALU.bypass)
                        e2f = scw.tile([128, TB], BF16, tag="wkv_e2f", bufs=2)
                        nc.vector.scalar_tensor_tensor(
                            e2f[:], k2, c_u[:, hp:hp + 1], r2,
                            ALU.mult, ALU.mult)
                        # rt = r*exp(e-qe), kt = k*exp(qe)  (bf16)
                        nc.vector.tensor_sub(e[:], e[:], qe[:])
                        eb = scw.tile([128, TB], BF16, tag="wkv_eb", bufs=2)
                        nc.scalar.activation(eb[:], e[:], AF.Exp)
                        rt = scw.tile([128, TB], BF16, tag="wkv_rt", bufs=2)
                        nc.vector.tensor_mul(rt[:], r2, eb[:])
                        ktb = scw.tile([128, TB], BF16, tag="wkv_eb", bufs=2,
                                       name="ktb")
                        nc.scalar.activation(ktb[:], qe[:], AF.Exp)
                        kt = scw.tile([128, TB], BF16, tag="wkv_kt", bufs=2)
                        nc.vector.tensor_mul(kt[:], k2, ktb[:])
                        ypb = sc.tile([128, TB], BF16, tag="wkv_ypb",
                                      bufs=2, name="ypb")
                        for cc in range(4):
                            cs = slice(128 * cc, 128 * (cc + 1))
                            qend = qe[:, 128 * cc + 127:128 * cc + 128]
                            pl2 = sc.tile([128, 1], F32, tag="wkv_pl")
                            nc.scalar.activation(pl2[:], qend, AF.Exp,
                                                 scale=-1.0)
                            kh = sc.tile([128, 128], BF16, tag="wkv_kh")
                            nc.vector.tensor_scalar(kh[:], kt[:, cs], pl2[:],
                                                    None, ALU.mult)
                            khT = ppb(128, 128)
                            nc.tensor.transpose(khT[:], kh[:], c_id[:])
                            khTs = sc.tile([128, 128], BF16, tag="wkv_khTs")
                            nc.scalar.activation(khTs[:], khT[:], AF.Copy)
                            ypk = sc.tile([128, 128], BF16, tag="wkv_ypk")
                            for hh in range(2):
                                h64 = slice(64 * hh, 64 * (hh + 1))
                                at = pp(128, 128)
                                nc.tensor.matmul(at[:], kt[h64, cs],
                                                 rt[h64, cs],
                                                 start=True, stop=True)
                                scol = pp(128, 1)
                                nc.tensor.matmul(scol[:], e2f[h64, cs],
                                                 ones_col[h64, :],
                                                 start=True, stop=True)
                                am = sc.tile([128, 128], BF16, tag="wkv_am")
                                nc.vector.tensor_mul(am[:], at[:], c_msk[:])
                                ydg = sc.tile([128, 64], BF16, tag="wkv_ydg")
                                nc.vector.tensor_scalar(ydg[:],
                                                        v2[:, cc, h64],
                                                        scol[:], None,
                                                        ALU.mult)
                                yp = pp(128, 64)
                                nc.tensor.matmul(yp[:], am[:], v2[:, cc, h64],
                                                 start=True, stop=False)
                                nc.tensor.matmul(yp[:], rt[h64, cs],
                                                 S2[h64, :],
                                                 start=False, stop=True)
                                sps = pp(64, 64)
                                nc.tensor.matmul(sps[:], khTs[:, h64],
                                                 v2[:, cc, h64],
                                                 start=True, stop=True)
                                nc.vector.scalar_tensor_tensor(
                                    S2[h64, :], S2[h64, :], pl2[h64, :],
                                    sps[:], ALU.mult, ALU.add)
                                nc.vector.tensor_add(ypk[:, h64], yp[:],
                                                     ydg[:])
                            ypT = ppb(128, 128)
                            nc.tensor.transpose(ypT[:], ypk[:], c_id[:])
                            nc.scalar.activation(ypb[:, cs], ypT[:], AF.Copy)
                        hy = nc.sync.dma_start(dstY[j], ypb[:])
                        if hp == 1 and "yb1_first" not in last_dma:
                            last_dma["yb1_first"] = hy
                        last_dma["yb_last"] = hy
                if hp == 0:
                    nc.gpsimd.collective_compute(
                        "AllToAll", ALU.bypass, replica_groups=RG,
                        ins=[a2b0_in[:]], outs=[a2b0_out[:]])
            nc.gpsimd.collective_compute(
                "AllToAll", ALU.bypass, replica_groups=RG,
                ins=[a2b1_in[:]], outs=[a2b1_out[:]])

            # ============ y assembly + GroupNorm (token domain) ============
            yt = big.tile([128, KC, TB], BF16, tag="mA", name="yt")
            psS, psSq = pp(32, TB), pp(32, TB)
            y_order = list(range(0, KC, 2)) + list(range(1, KC, 2))
            for i, m in enumerate(y_order):
                src = a2b0_out if m % 2 == 0 else a2b1_out
                hy = nc.scalar.dma_start(yt[:, m, :], src[m // 2])
                anchor = "yb1_first" if m % 2 == 0 else "yb_last"
                add_dep_helper(hy.ins, last_dma[anchor].ins, sync=False,
                               reason="dma-lane order")
                sq = sc.tile([128, TB], BF16, tag="g1")
                nc.scalar.activation(sq[:], yt[:, m, :], AF.Square)
                st, sp = (i == 0), (i == KC - 1)
                nc.tensor.matmul(psS[:], c_gsel[:, m, :], yt[:, m, :],
                                 start=st, stop=sp)
                nc.tensor.matmul(psSq[:], c_gsel[:, m, :], sq[:],
                                 start=st, stop=sp)
            gst = lnp.tile([32, 2 * TB], F32, tag="ln_stats")
            gmean, ginv = gst[:, 0:TB], gst[:, TB:]
            nc.scalar.activation(gmean[:], psS[:], AF.Copy, scale=1.0 / 64)
            nc.scalar.activation(ginv[:], psSq[:], AF.Copy, scale=1.0 / 64)
            gvar = lnp.tile([32, TB], F32, tag="ln_work")
            nc.vector.tensor_mul(gvar[:], gmean[:], gmean[:])
            nc.vector.tensor_sub(gvar[:], ginv[:], gvar[:])
            nc.scalar.activation(ginv[:], gvar[:], AF.Sqrt, bias=EPS_LNX)
            nc.vector.reciprocal(ginv[:], ginv[:])
            gstb = lnp.tile([32, 2 * TB], BF16, tag="ln_stb")
            nc.vector.tensor_copy(gstb[:], gst[:])
            for m in range(KC):
                bcM = pp(128, TB)
                nc.tensor.matmul(bcM[:], c_bsel[:, m, :], gstb[:, 0:TB],
                                 start=True, stop=True)
                bcI = pp(128, TB)
                nc.tensor.matmul(bcI[:], c_bsel[:, m, :], gstb[:, TB:],
                                 start=True, stop=True)
                yn = sc.tile([128, TB], BF16, tag="g1")
                nc.vector.tensor_sub(yn[:], yt[:, m, :], bcM[:])
                nc.vector.tensor_mul(yn[:], yn[:], bcI[:])
                nc.vector.tensor_scalar(yn[:], yn[:], c_lnx[:, m, 0:1],
                                        c_lnx[:, m, 1:2], ALU.mult, ALU.add)
                nc.vector.tensor_mul(yt[:, m, :], yn[:], gsb[:, m, :])

            # ============ att out proj + residual + LN2 stats ============
            x2b = big.tile([128, KC, TB], BF16, tag="xb", name="x2b")
            psA3, psA4 = pp(1, TB), pp(1, TB)
            for m in range(KC):
                wt = wstr.tile([128, KC, 128], BF16, tag="wstream", bufs=2)
                nc.sync.dma_start(wt[:], Wp["o"][m])
                pt = pp(128, TB)
                for k in range(KC):
                    nc.tensor.matmul(pt[:], wt[:, k, :], yt[:, k, :],
                                     start=(k == 0), stop=(k == KC - 1))
                x2t = sc.tile([128, TB], F32, tag="g3", bufs=2)
                xin = sc.tile([128, TB], F32, tag="g4", bufs=2)
                nc.sync.dma_start(xin[:], xT[128 * m:128 * (m + 1), 1:TB + 1])
                nc.vector.tensor_add(x2t[:], pt[:], xin[:])
                nc.sync.dma_start(x2d[128 * m:128 * (m + 1), :], x2t[:])
                nc.scalar.activation(x2b[:, m, :], x2t[:], AF.Copy)
                sq = sc.tile([128, TB], BF16, tag="g1")
                nc.scalar.activation(sq[:], x2b[:, m, :], AF.Square)
                st, sp = (m == 0), (m == KC - 1)
                nc.tensor.matmul(psA3[:], ones_col[:], x2b[:, m, :],
                                 start=st, stop=sp)
                nc.tensor.matmul(psA4[:], ones_col[:], sq[:],
                                 start=st, stop=sp)

            # ---- ln2 normalize ----
            stats2 = lnp.tile([1, 2 * TB], F32, tag="ln_stats")
            mean2, msq2 = stats2[:, 0:TB], stats2[:, TB:]
            nc.scalar.activation(mean2[:], psA3[:], AF.Copy, scale=1.0 / C)
            nc.scalar.activation(msq2[:], psA4[:], AF.Copy, scale=1.0 / C)
            wk2 = lnp.tile([1, TB], F32, tag="ln_work")
            nc.vector.tensor_mul(wk2[:], mean2[:], mean2[:])
            nc.vector.tensor_sub(wk2[:], msq2[:], wk2[:])
            nc.scalar.activation(wk2[:], wk2[:], AF.Sqrt, bias=EPS_LN)
            nc.vector.reciprocal(wk2[:], wk2[:])
            stb2 = lnp.tile([1, 2 * TB], BF16, tag="ln_stb")
            nc.vector.tensor_copy(stb2[:, 0:TB], mean2[:])
            nc.vector.tensor_copy(stb2[:, TB:], wk2[:])
            bmp3, bip3 = pp(128, TB), pp(128, TB)
            nc.tensor.matmul(bmp3[:], ones_row[:], stb2[:, 0:TB], start=True,
                             stop=True)
            nc.tensor.matmul(bip3[:], ones_row[:], stb2[:, TB:], start=True,
                             stop=True)
            bc2 = lnp.tile([128, 2 * TB], BF16, tag="ln_bc")
            nc.vector.tensor_copy(bc2[:, 0:TB], bmp3[:])
            nc.vector.tensor_copy(bc2[:, TB:], bip3[:])
            # boundary column first so the AllGather overlaps the
            # full-width normalize pass below
            for k in range(KC):
                tb1 = sc.tile([128, 1], BF16, tag="bnd")
                nc.vector.tensor_sub(tb1[:], x2b[:, k, TB - 1:TB],
                                     bc2[:, TB - 1:TB])
                nc.vector.tensor_mul(tb1[:], tb1[:], bc2[:, 2 * TB - 1:])
                nc.vector.tensor_scalar(ht[:, k, TB:TB + 1], tb1[:],
                                        c_ln2[:, k, 0:1], c_ln2[:, k, 1:2],
                                        ALU.mult, ALU.add)
            last_dma["ag"] = nc.sync.dma_start(
                ag_in[0:1, :].rearrange("o (k p) -> p k o", p=128),
                ht[:, :, TB:TB + 1])
            nc.gpsimd.collective_compute(
                "AllGather", ALU.bypass, replica_groups=RG,
                ins=[ag_in[:]], outs=[ag_out[:]])

            for k in range(KC):
                t = sc.tile([128, TB], BF16, tag="e2")
                nc.vector.tensor_sub(t[:], x2b[:, k, :], bc2[:, 0:TB])
                nc.vector.tensor_mul(t[:], t[:], bc2[:, TB:])
                nc.vector.tensor_scalar(ht[:, k, 1:TB + 1], t[:],
                                        c_ln2[:, k, 0:1], c_ln2[:, k, 1:2],
                                        ALU.mult, ALU.add)

            for q in range(4):
                agp = sc.tile([NCORE, TB], BF16, tag="agp", bufs=1)
                ha = nc.scalar.dma_start(agp[:],
                                         ag_out[:, 512 * q:512 * (q + 1)])
                add_dep_helper(ha.ins, last_dma["ag"].ins, sync=False,
                               reason="dma-lane order")
                hp_ = pp(1, TB)
                nc.tensor.matmul(hp_[:], c_sel[:], agp[:],
                                 start=True, stop=True)
                hrow = sc.tile([1, TB], BF16, tag="hrow")
                nc.vector.tensor_copy(hrow[:], hp_[:])
                for mm in range(4):
                    m = 4 * q + mm
                    nc.sync.dma_start(ht[:, m, 0:1],
                                      hrow[0:1, 128 * mm:128 * (mm + 1)])
            for k in range(KC):
                x0 = sc.tile([128, 1], BF16, tag="bnd")
                nc.vector.tensor_sub(x0[:], ht[:, k, 0:1], ht[:, k, 1:2])
                nc.vector.scalar_tensor_tensor(
                    xx[:, k, 0:1], x0[:], c_cm[:, k, 0:1], ht[:, k, 1:2],
                    ALU.mult, ALU.add)
                nc.vector.scalar_tensor_tensor(
                    gsb[:, k, 0:1], x0[:], c_cm[:, k, 1:2], ht[:, k, 1:2],
                    ALU.mult, ALU.add)

            for k in range(KC):
                # columns 1..TB-1 need no halo; they overlap the AllGather
                xx2 = sc.tile([128, TB], BF16, tag="g1")
                nc.vector.tensor_sub(xx2[:, 1:], ht[:, k, 1:TB],
                                     ht[:, k, 2:TB + 1])
                nc.vector.scalar_tensor_tensor(
                    xx[:, k, 1:], xx2[:, 1:], c_cm[:, k, 0:1],
                    ht[:, k, 2:TB + 1], ALU.mult, ALU.add)        # xk2
                nc.vector.scalar_tensor_tensor(
                    gsb[:, k, 1:], xx2[:, 1:], c_cm[:, k, 1:2],
                    ht[:, k, 2:TB + 1], ALU.mult, ALU.add)        # xr2

            # ============ FFN ============
            kfA = big.tile([128, KC, TB], BF16, tag="mB")
            kfB = big.tile([128, KC, TB], BF16, tag="mA")
            kfC = big.tile([128, KC, TB], BF16, tag="ht")
            kfD = big.tile([128, 8, TB], BF16, tag="xb")

            def kf_view(i):
                if i < KC:
                    return kfA[:, i, :]
                if i < 32:
                    return kfB[:, i - 16, :]
                return kfC[:, i - 32, :] if i < 48 else kfD[:, i - 48, :]

            for mf in range(KF):
                wt = wstr.tile([128, KC, 128], BF16, tag="wstream", bufs=2)
                nc.sync.dma_start(wt[:], Wck_p[mf])
                pt = pp(128, TB)
                for k in range(KC):
                    nc.tensor.matmul(pt[:], wt[:, k, :], xx[:, k, :],
                                     start=(k == 0), stop=(k == KC - 1))
                rl = sc.tile([128, TB], BF16, tag="g1")
                nc.vector.tensor_scalar(rl[:], pt[:], 0.0, None, ALU.max)
                nc.scalar.activation(kf_view(mf), rl[:], AF.Square)

            for m in range(KC):
                ptu = pp(128, TB)
                for q in range(4):
                    wcv = wstr.tile([128, 14, 128], BF16, tag="wcv_s", bufs=2)
                    nc.sync.dma_start(wcv[:],
                                      Wcv_p[m, :, q * 14:(q + 1) * 14, :])
                    for kk in range(14):
                        ki = q * 14 + kk
                        nc.tensor.matmul(ptu[:], wcv[:, kk, :], kf_view(ki),
                                         start=(ki == 0), stop=(ki == KF - 1))
                wt = wstr.tile([128, KC, 128], BF16, tag="wstream", bufs=2)
                nc.sync.dma_start(wt[:], Wp["cr"][m])
                pts = pp(128, TB)
                for k in range(KC):
                    nc.tensor.matmul(pts[:], wt[:, k, :], gsb[:, k, :],
                                     start=(k == 0), stop=(k == KC - 1))
                ssb = sc.tile([128, TB], BF16, tag="g2")
                nc.scalar.activation(ssb[:], pts[:], AF.Sigmoid)
                ot = sc.tile([128, TB], F32, tag="g3", bufs=2)
                x2in = sc.tile([128, TB], F32, tag="g4", bufs=2)
                nc.sync.dma_start(x2in[:],
                                  x2d[128 * m:128 * (m + 1), :])
                nc.vector.tensor_mul(ot[:], ptu[:], ssb[:])
                nc.vector.tensor_add(ot[:], ot[:], x2in[:])
                nc.sync.dma_start(outT[128 * m:128 * (m + 1), :], ot[:])

    nc.compile()
    return nc


_CACHE = {}


def _get_program():
    if "nc" not in _CACHE:
        _CACHE["nc"] = build_program()
    return _CACHE["nc"]


def _pret4(w):
    """(Cin, Cout) -> (Cout/128, 128, Cin/128, 128): [m,p,k,f] = w[128k+p, 128m+f]
    so each m-tile is one fully-contiguous per-partition DMA."""
    ci, co = w.shape
    return np.ascontiguousarray(
        w.reshape(ci // 128, 128, co // 128, 128).transpose(2, 1, 0, 3))


def _shard_inputs(inp):
    f32 = np.float32
    x = np.asarray(inp["x"], f32)
    bf = lambda a: np.asarray(a, f32).astype(NP_BF16)

    maa_w2 = np.asarray(inp["maa_w2"], f32)
    w2p = np.zeros((32, 5, KC, 128), f32)
    for i in range(5):
        for m in range(KC):
            w2p[:, i, m, :] = maa_w2[i][:, 128 * m:128 * (m + 1)]
    td_w2 = np.asarray(inp["td_w2"], f32)
    td2p = td_w2.reshape(64, KC, 128).copy()
    for m in range(KC):
        td2p[:, m, :] = td_w2[:, 128 * m:128 * (m + 1)]

    gsel = np.zeros((128, KC, 32), f32)
    bsel = np.zeros((32, KC, 128), f32)
    for p in range(128):
        for k in range(KC):
            gsel[p, k, 2 * k + p // 64] = 1.0
            bsel[2 * k + p // 64, k, p] = 1.0

    shared = {
        "ln1_wb": np.stack([inp["ln1_w"], inp["ln1_b"]], 1).astype(f32),
        "ln2_wb": np.stack([inp["ln2_w"], inp["ln2_b"]], 1).astype(f32),
        "lnx_wb": np.stack([inp["lnx_w"], inp["lnx_b"]], 1).astype(f32),
        "tm_maaT": np.asarray(inp["tm_maa"], f32).T.copy(),
        "cm_maaT": np.asarray(inp["cm_maa"], f32).T.copy(),
        "td_col": np.asarray(inp["time_decay"], f32).reshape(C, 1),
        "ident": np.eye(128, dtype=f32).astype(NP_BF16),
        "mask_su": np.triu(np.ones((128, 128), f32), 1).astype(NP_BF16),
        "gsel": gsel.reshape(128, KC * 32).astype(NP_BF16),
        "bsel": bsel.reshape(32, KC * 128).astype(NP_BF16),
        "maa_w1": bf(inp["maa_w1"]),
        "maa_w2p": w2p.astype(NP_BF16),
        "td_w1": bf(inp["td_w1"]),
        "td_w2p": td2p.astype(NP_BF16),
        "Wr_p": bf(_pret4(np.asarray(inp["Wr"], f32))),
        "Wk_p": bf(_pret4(np.asarray(inp["Wk"], f32))),
        "Wg_p": bf(_pret4(np.asarray(inp["Wg"], f32))),
        "Wo_p": bf(_pret4(np.asarray(inp["Wo"], f32))),
        "Wcr_p": bf(_pret4(np.asarray(inp["Wcr"], f32))),
        "Wv": bf(inp["Wv"]),
        "Wck_p": bf(_pret4(np.asarray(inp["Wck"], f32))),
        "Wcv_p": bf(_pret4(np.asarray(inp["Wcv"], f32))),
    }
    u = np.asarray(inp["time_faaaa"], f32).reshape(C)

    in_maps = []
    for c in range(NCORE):
        b, blk = c // 4, c % 4
        ts = blk * TB
        xe = np.zeros((C, TB + 1), f32)
        xe[:, 1:] = x[b, ts:ts + TB].T
        if blk > 0:
            xe[:, 0] = x[b, ts - 1]
        ul = u[LCH * c:LCH * (c + 1)].reshape(2, 128).T.copy()
        sel = np.zeros((NCORE, 1), NP_BF16)
        if blk > 0:
            sel[c - 1, 0] = 1.0
        m = dict(shared)
        m.update({
            "xT": xe,
            "halo_mask": np.full((128, 1), 1.0 if blk > 0 else 0.0, f32),
            "sel_prev": sel,
            "u_loc": ul,
        })
        in_maps.append(m)
    return in_maps


def run(inputs, trace=False):
    nc = _get_program()
    in_maps = _shard_inputs(inputs)
    res = bass_utils.run_bass_kernel_spmd(
        nc, in_maps, core_ids=list(range(NCORE)), trace=trace)
    x = np.asarray(inputs["x"], np.float32)
    out = np.empty_like(x)
    for c in range(NCORE):
        b, blk = c // 4, c % 4
        out[b, blk * TB:(blk + 1) * TB, :] = np.asarray(
            res.results[c]["out"], np.float32).T
    return out, res.exec_time_ns


def kernel(**inputs):
    out, _ = run(inputs)
    return out


if __name__ == "__main__":
    build_program()
    print("build ok")


# revision 26
# speedup vs baseline: 1.8057x; 1.7791x over previous
"""RWKV6 block (nn_Block_14602888806424) on 8 Trainium2 NeuronCores.

Token-sharded (sequence-parallel): each core owns 512 tokens (B=2 x 4
blocks); matmuls/LNs/mixing are token-local in channel-major layout.
r/k/w/v are redistributed head-sharded via AllToAll around the chunked
(L=128) WKV linear-attention scan (4 heads/core); a second pair of
AllToAlls (split by head-pair half) returns raw y; GroupNorm is applied
in the token-sharded domain with PE-based group stats. A small AllGather
carries the 1-token boundary halo for the second token-shift.

Perf structure: all collective payloads are bf16; the forward exchange is
split (v first, then r/k/w even channel-halves, then odd) so projection
matmuls overlap collective time; DMA loads that wait on collective
results issue from the ACT HWDGE queue so they cannot head-of-line-block
the SP DMA queue; weights are host-pre-tiled so every streamed weight
tile is a contiguous per-partition DMA; matmul operands are bf16 (fp32
matmuls run at 1/4 rate).
"""

import sys
import numpy as np

sys.path.insert(0, "/opt/trn_rl_repo")

import concourse.bass as bass
import concourse.bacc as bacc
import concourse.mybir as mybir
import concourse.tile as tile
from concourse.tile_rust import add_dep_helper
from concourse import bass_utils

F32 = mybir.dt.float32
BF16 = mybir.dt.bfloat16
NP_BF16 = mybir.dt.np(BF16)
AF = mybir.ActivationFunctionType
ALU = mybir.AluOpType

B, T, C, H, N, FF = 2, 2048, 2048, 32, 64, 7168
D_MIX, D_DECAY = 32, 64
EPS_LN = 1e-5
EPS_LNX = 1e-5 * 8.0**2
NCORE = 8
TB = 512
KC = C // 128          # 16
KF = FF // 128         # 56
LCH = 256              # channels per core (4 heads)
RG = [list(range(NCORE))]


def build_program():
    nc = bacc.Bacc("TRN2", target_bir_lowering=False, debug=False,
                   num_devices=NCORE, enable_asserts=False)

    def din(name, shape, dt=F32):
        return nc.dram_tensor(name, list(shape), dt, kind="ExternalInput").ap()

    xT = din("xT", (C, TB + 1))
    halo_mask = din("halo_mask", (128, 1))
    sel_prev = din("sel_prev", (NCORE, 1), BF16)
    u_loc = din("u_loc", (128, 2))
    lnx_wb = din("lnx_wb", (C, 2))
    ln1_wb = din("ln1_wb", (C, 2))
    ln2_wb = din("ln2_wb", (C, 2))
    tm_maaT = din("tm_maaT", (C, 6))
    cm_maaT = din("cm_maaT", (C, 2))
    td_col = din("td_col", (C, 1))
    ident = din("ident", (128, 128), BF16)
    mask_su = din("mask_su", (128, 128), BF16)
    gsel = din("gsel", (128, KC * 32), BF16)
    bsel = din("bsel", (64, KC * 128), BF16)
    maa_w1 = din("maa_w1", (C, 5 * D_MIX), BF16)
    maa_w2p = din("maa_w2p", (32, 5, KC, 128), BF16)
    td_w1 = din("td_w1", (C, D_DECAY), BF16)
    td_w2p = din("td_w2p", (64, KC, 128), BF16)
    Wp = {k: din(f"W{k}_p", (KC, 128, KC, 128), BF16)
          for k in ["r", "k", "g", "cr"]}
    Wo_kp = din("Wo_kp", (KC, 128, KC, 128), BF16)
    Wv = din("Wv", (C, C), BF16)
    Wck_p = din("Wck_p", (KC, 128, KF, 128), BF16)
    Wcv_p = din("Wcv_p", (KC, 128, KF, 128), BF16)

    outT = nc.dram_tensor("out", [C, TB], F32, kind="ExternalOutput").ap()

    with tile.TileContext(nc) as tc:
        import contextlib
        with contextlib.ExitStack() as ctx:
            dram = ctx.enter_context(tc.tile_pool(name="dram", bufs=1,
                                                  space="DRAM"))
            cpool = ctx.enter_context(tc.tile_pool(name="const", bufs=1))
            big = ctx.enter_context(tc.tile_pool(name="big", bufs=1))
            wstr = ctx.enter_context(tc.tile_pool(name="wstr", bufs=3))
            sc = ctx.enter_context(tc.tile_pool(name="scratch", bufs=2))
            scw = ctx.enter_context(tc.tile_pool(name="scw", bufs=1))
            lnp = ctx.enter_context(tc.tile_pool(name="lnp", bufs=1))
            ps = ctx.enter_context(
                tc.tile_pool(name="psum", bufs=8, space="PSUM"))

            def pp(p_, f_):
                return ps.tile([p_, f_], F32, tag="pp", name="pp")

            def ppb(p_, f_):
                return ps.tile([p_, f_], BF16, tag="pp", name="ppb")

            # ---- DRAM internals (all collective payloads bf16) ----
            a2aA_in = dram.tile([NCORE, 3, 128, TB], BF16, tag="a2aA_in")
            a2aA_out = dram.tile([NCORE, 3, 128, TB], BF16, tag="a2aA_out")
            a2aB_in = dram.tile([NCORE, 3, 128, TB], BF16, tag="a2aB_in")
            a2aB_out = dram.tile([NCORE, 3, 128, TB], BF16, tag="a2aB_out")
            a2v_in = dram.tile([NCORE, TB, LCH], BF16, tag="a2v_in")
            a2v_out = dram.tile([NCORE, TB, LCH], BF16, tag="a2v_out")
            a2b0_in = dram.tile([NCORE, 128, TB], BF16, tag="a2b0_in")
            a2b0_out = dram.tile([NCORE, 128, TB], BF16, tag="a2b0_out")
            a2b1_in = dram.tile([NCORE, 128, TB], BF16, tag="a2b1_in")
            a2b1_out = dram.tile([NCORE, 128, TB], BF16, tag="a2b1_out")
            ag_in = dram.tile([1, C], BF16, tag="ag_in")
            ag_out = dram.tile([NCORE, C], BF16, tag="ag_out",
                               addr_space="Shared")
            x2d = dram.tile([C, TB], F32, tag="x2d")

            # ---- constants ----
            def cload(name, src, shape, dt=F32, rearr=None):
                t = cpool.tile(list(shape), dt, tag=name)
                nc.sync.dma_start(t[:], src if rearr is None
                                  else src.rearrange(rearr, p=128))
                return t

            c_ln1 = cload("c_ln1", ln1_wb, (128, KC, 2), F32, "(k p) f -> p k f")
            c_ln2 = cload("c_ln2", ln2_wb, (128, KC, 2), F32, "(k p) f -> p k f")
            c_lnx = cload("c_lnx", lnx_wb, (128, KC, 2), F32, "(k p) f -> p k f")
            c_tm = cload("c_tm", tm_maaT, (128, KC, 6), F32, "(k p) f -> p k f")
            c_cm = cload("c_cm", cm_maaT, (128, KC, 2), F32, "(k p) f -> p k f")
            c_td = cload("c_td", td_col, (128, KC, 1), F32, "(k p) f -> p k f")
            c_hm = cload("c_hm", halo_mask, (128, 1))
            c_sel = cload("c_sel", sel_prev, (NCORE, 1), BF16)
            c_u = cload("c_u", u_loc, (128, 2))
            c_id = cload("c_id", ident, (128, 128), BF16)
            c_msk = cload("c_msk", mask_su, (128, 128), BF16)
            c_gsel = cload("c_gsel", gsel, (128, KC, 32), BF16)
            c_bsel = cload("c_bsel", bsel, (64, KC, 128), BF16)
            c_w1 = cload("c_w1", maa_w1, (128, KC, 5 * D_MIX), BF16,
                         "(k p) f -> p k f")
            c_td1 = cload("c_td1", td_w1, (128, KC, D_DECAY), BF16,
                          "(k p) f -> p k f")
            c_td2 = cload("c_td2", td_w2p, (64, KC, 128), BF16)
            ones_col = cpool.tile([128, 1], BF16, tag="ones_col")
            nc.vector.memset(ones_col[:], 1.0)
            ones_row = cpool.tile([1, 128], BF16, tag="ones_row")
            nc.vector.memset(ones_row[:], 1.0)
            for _cv in (EPS_LN, EPS_LNX):
                cvt = cpool.tile([128, 1], F32, tag=f"cv{_cv}", name="cvt")
                nc.vector.memset(cvt[:], _cv)
                nc.const_aps.aps[(F32, _cv)] = cvt[:]

            # ---- persistent SBUF ----
            xb = big.tile([128, KC, TB + 1], BF16, tag="xb")
            ht = big.tile([128, KC, TB + 1], BF16, tag="ht")
            xx = big.tile([128, KC, TB], BF16, tag="xx")      # later xk2
            gsb = big.tile([128, KC, TB], BF16, tag="gsb")    # later xr2

            # ============ LN1 over TB+1 cols (src resident in xb) ============
            psA, psB = pp(1, TB), pp(1, 1)
            psA2, psB2 = pp(1, TB), pp(1, 1)
            for k in range(KC):
                nc.gpsimd.dma_start(xb[:, k, :], xT[128 * k:128 * (k + 1), :])
                sq = sc.tile([128, TB + 1], BF16, tag="e2")
                nc.scalar.activation(sq[:], xb[:, k, :], AF.Square)
                st, sp = (k == 0), (k == KC - 1)
                nc.tensor.matmul(psA[:], ones_col[:], xb[:, k, 0:TB],
                                 start=st, stop=sp)
                nc.tensor.matmul(psB[:], ones_col[:], xb[:, k, TB:TB + 1],
                                 start=st, stop=sp)
                nc.tensor.matmul(psA2[:], ones_col[:], sq[:, 0:TB],
                                 start=st, stop=sp)
                nc.tensor.matmul(psB2[:], ones_col[:], sq[:, TB:TB + 1],
                                 start=st, stop=sp)
            stats = lnp.tile([1, 2 * (TB + 1)], F32, tag="ln_stats")
            mean, msq = stats[:, 0:TB + 1], stats[:, TB + 1:]
            nc.scalar.activation(mean[:, 0:TB], psA[:], AF.Copy, scale=1.0 / C)
            nc.scalar.activation(mean[:, TB:TB + 1], psB[:], AF.Copy,
                                 scale=1.0 / C)
            nc.scalar.activation(msq[:, 0:TB], psA2[:], AF.Copy, scale=1.0 / C)
            nc.scalar.activation(msq[:, TB:TB + 1], psB2[:], AF.Copy,
                                 scale=1.0 / C)
            wk = lnp.tile([1, TB + 1], F32, tag="ln_work")
            nc.vector.tensor_mul(wk[:], mean[:], mean[:])
            nc.vector.tensor_sub(wk[:], msq[:], wk[:])
            nc.scalar.activation(wk[:], wk[:], AF.Sqrt, bias=EPS_LN)
            nc.vector.reciprocal(wk[:], wk[:])
            stb = lnp.tile([1, 2 * (TB + 1)], BF16, tag="ln_stb")
            nc.vector.tensor_copy(stb[:, 0:TB + 1], mean[:])
            nc.vector.tensor_copy(stb[:, TB + 1:], wk[:])
            bmp, bmp2 = pp(128, TB), pp(128, 1)
            bip, bip2 = pp(128, TB), pp(128, 1)
            nc.tensor.matmul(bmp[:], ones_row[:], stb[:, 0:TB],
                             start=True, stop=True)
            nc.tensor.matmul(bmp2[:], ones_row[:], stb[:, TB:TB + 1],
                             start=True, stop=True)
            nc.tensor.matmul(bip[:], ones_row[:], stb[:, TB + 1:2 * TB + 1],
                             start=True, stop=True)
            nc.tensor.matmul(bip2[:], ones_row[:], stb[:, 2 * TB + 1:],
                             start=True, stop=True)
            bc = lnp.tile([128, 2 * (TB + 1)], BF16, tag="ln_bc")
            bm, bi = bc[:, 0:TB + 1], bc[:, TB + 1:]
            nc.vector.tensor_copy(bm[:, 0:TB], bmp[:])
            nc.vector.tensor_copy(bm[:, TB:TB + 1], bmp2[:])
            nc.vector.tensor_copy(bi[:, 0:TB], bip[:])
            nc.vector.tensor_copy(bi[:, TB:TB + 1], bip2[:])
            for k in range(KC):
                tn = sc.tile([128, TB + 1], BF16, tag="e2")
                nc.vector.tensor_sub(tn[:], xb[:, k, :], bm[:])
                nc.vector.tensor_mul(tn[:], tn[:], bi[:])
                d = ht[:, k, :]
                nc.vector.tensor_scalar(d, tn[:], c_ln1[:, k, 0:1],
                                        c_ln1[:, k, 1:2], ALU.mult, ALU.add)
                nc.vector.tensor_scalar(d[:, 0:1], d[:, 0:1], c_hm[:],
                                        None, ALU.mult)
                nc.vector.tensor_sub(xx[:, k, :], ht[:, k, 0:TB],
                                     ht[:, k, 1:TB + 1])

            # ============ maa ============
            aps1, aps2 = pp(128, TB), pp(32, TB)
            for k in range(KC):
                xxx = sc.tile([128, TB], BF16, tag="xxx")
                nc.vector.scalar_tensor_tensor(
                    xxx[:], xx[:, k, :], c_tm[:, k, 0:1], ht[:, k, 1:TB + 1],
                    ALU.mult, ALU.add)
                nc.tensor.matmul(aps1[:], c_w1[:, k, 0:128], xxx[:],
                                 start=(k == 0), stop=(k == KC - 1))
                nc.tensor.matmul(aps2[:], c_w1[:, k, 128:160], xxx[:],
                                 start=(k == 0), stop=(k == KC - 1))
            aTs = [cpool.tile([32, TB], BF16, tag=f"aT{i}", name="aTs")
                   for i in range(5)]
            for i in range(4):
                nc.scalar.activation(aTs[i][:], aps1[32 * i:32 * (i + 1), :],
                                     AF.Tanh)
            nc.scalar.activation(aTs[4][:], aps2[0:32, :], AF.Tanh)

            def make_mix(i, tag):
                mt = big.tile([128, KC, TB], BF16, tag=tag, name="mixbuf")
                for k in range(KC):
                    w2s = wstr.tile([32, 128], BF16, tag="w2s")
                    nc.sync.dma_start(w2s[:], maa_w2p[:, i, k, :])
                    mp = pp(128, TB)
                    nc.tensor.matmul(mp[:], w2s[:], aTs[i][:],
                                     start=True, stop=True)
                    mpc = sc.tile([128, TB], BF16, tag="mpc")
                    nc.scalar.activation(mpc[:], mp[:], AF.Copy)
                    t = sc.tile([128, TB], BF16, tag="g1")
                    nc.vector.scalar_tensor_tensor(
                        t[:], mpc[:], c_tm[:, k, i + 1:i + 2], xx[:, k, :],
                        ALU.add, ALU.mult)
                    nc.vector.tensor_add(mt[:, k, :], t[:],
                                         ht[:, k, 1:TB + 1])
                return mt

            last_dma = {}

            def proj_cm(wp_ap, sink, src_view, ms):
                for m in ms:
                    wt = wstr.tile([128, KC, 128], BF16, tag="wstream", bufs=2)
                    last_dma["wt"] = nc.sync.dma_start(wt[:], wp_ap[m])
                    pt = pp(128, TB)
                    for k in range(KC):
                        nc.tensor.matmul(pt[:], wt[:, k, :], src_view(k),
                                         start=(k == 0), stop=(k == KC - 1))
                    sink(m, pt)

            def sink_a2a(idx):
                def s(m, pt):
                    st = sc.tile([128, TB], BF16, tag="g2")
                    nc.vector.tensor_copy(st[:], pt[:])
                    buf = a2aA_in if m % 2 == 0 else a2aB_in
                    nc.sync.dma_start(buf[m // 2, idx], st[:])
                return s

            # ---- v projection first: its A2A starts before everything ----
            xv_t = make_mix(2, "mA")
            for cc in range(4):
                pvs = [pp(128, TB) for _ in range(4)]
                for k in range(KC):
                    wv_t = wstr.tile([128, TB], BF16, tag="wv_s", bufs=2)
                    nc.sync.dma_start(
                        wv_t[:], Wv[128 * k:128 * (k + 1),
                                    512 * cc:512 * (cc + 1)])
                    for t4 in range(4):
                        nc.tensor.matmul(
                            pvs[t4][:], xv_t[:, k, 128 * t4:128 * (t4 + 1)],
                            wv_t[:], start=(k == 0), stop=(k == KC - 1))
                for t4 in range(4):
                    st = sc.tile([128, TB], BF16, tag="g2")
                    nc.vector.tensor_copy(st[:], pvs[t4][:])
                    for half in range(2):
                        nc.sync.dma_start(
                            a2v_in[2 * cc + half, 128 * t4:128 * (t4 + 1), :],
                            st[:, 256 * half:256 * (half + 1)])

            nc.gpsimd.collective_compute(
                "AllToAll", ALU.bypass, replica_groups=RG,
                ins=[a2v_in[:]], outs=[a2v_out[:]])

            # ---- r/k/w projections, even channel-halves then odd ----
            EV = list(range(0, KC, 2))
            OD = list(range(1, KC, 2))
            xr_t = make_mix(3, "mA")
            xk_t = make_mix(1, "mB")
            # w-decay mix is transient: consumed chunk-by-chunk into t1p
            t1p = pp(64, TB)
            for k in range(KC):
                w2s = wstr.tile([32, 128], BF16, tag="w2s")
                nc.sync.dma_start(w2s[:], maa_w2p[:, 0, k, :])
                mp = pp(128, TB)
                nc.tensor.matmul(mp[:], w2s[:], aTs[0][:],
                                 start=True, stop=True)
                mpc = sc.tile([128, TB], BF16, tag="mpc")
                nc.scalar.activation(mpc[:], mp[:], AF.Copy)
                xwk = sc.tile([128, TB], BF16, tag="xxx")
                nc.vector.scalar_tensor_tensor(
                    xwk[:], mpc[:], c_tm[:, k, 1:2], xx[:, k, :],
                    ALU.add, ALU.mult)
                nc.vector.tensor_add(xwk[:], xwk[:], ht[:, k, 1:TB + 1])
                nc.tensor.matmul(t1p[:], c_td1[:, k, :], xwk[:],
                                 start=(k == 0), stop=(k == KC - 1))
            t1 = cpool.tile([64, TB], BF16, tag="t1")
            nc.scalar.activation(t1[:], t1p[:], AF.Tanh)

            def w_half(ms):
                for m in ms:
                    wp2 = pp(128, TB)
                    nc.tensor.matmul(wp2[:], c_td2[:, m, :], t1[:],
                                     start=True, stop=True)
                    st = sc.tile([128, TB], BF16, tag="g2")
                    nc.vector.tensor_scalar(st[:], wp2[:], c_td[:, m, 0:1],
                                            None, ALU.add)
                    buf = a2aA_in if m % 2 == 0 else a2aB_in
                    nc.sync.dma_start(buf[m // 2, 2], st[:])

            proj_cm(Wp["r"], sink_a2a(0), lambda k: xr_t[:, k, :], EV)
            proj_cm(Wp["k"], sink_a2a(1), lambda k: xk_t[:, k, :], EV)
            w_half(EV)

            nc.gpsimd.collective_compute(
                "AllToAll", ALU.bypass, replica_groups=RG,
                ins=[a2aA_in[:]], outs=[a2aA_out[:]])

            proj_cm(Wp["r"], sink_a2a(0), lambda k: xr_t[:, k, :], OD)
            proj_cm(Wp["k"], sink_a2a(1), lambda k: xk_t[:, k, :], OD)
            w_half(OD)

            nc.gpsimd.collective_compute(
                "AllToAll", ALU.bypass, replica_groups=RG,
                ins=[a2aB_in[:]], outs=[a2aB_out[:]])

            # ---- g projection (overlaps the odd-half collective) ----
            xg_t = make_mix(4, "mB")

            def sink_g(m, pt):
                nc.scalar.activation(gsb[:, m, :], pt[:], AF.Silu)
            proj_cm(Wp["g"], sink_g, lambda k: xg_t[:, k, :], list(range(KC)))

            # ============ WKV (chunked linear attention) ============
            # loads that wait on collective outputs go through nc.scalar
            # (ACT HWDGE) so they can't head-of-line-block the SP queue.
            for hp in range(2):
                srcRKW = a2aA_out if hp == 0 else a2aB_out
                dstY = a2b0_in if hp == 0 else a2b1_in
                S2s = {}
                for b in range(2):
                    S2s[b] = cpool.tile([128, 64], BF16, tag=f"S_{hp}_{b}",
                                        name="S2t")
                    nc.vector.memset(S2s[b][:], 0.0)
                for jb in range(4):
                    for b in range(2):
                        j = 4 * b + jb
                        S2 = S2s[b]
                        hs = slice(128 * hp, 128 * (hp + 1))
                        rkw = scw.tile([128, 3, TB], BF16, tag="wkv_rkw",
                                       bufs=2)
                        v2 = scw.tile([128, 4, 128], BF16, tag="wkv_v", bufs=2)
                        h1 = nc.scalar.dma_start(
                            rkw[:], srcRKW[j].rearrange("c p t -> p c t"))
                        h2 = nc.scalar.dma_start(
                            v2[:], a2v_out[j, :, hs]
                            .rearrange("(cc p) c -> p cc c", p=128))
                        add_dep_helper(h1.ins, last_dma["wt"].ins, sync=False,
                                       reason="dma-lane order")
                        add_dep_helper(h2.ins, last_dma["wt"].ins, sync=False,
                                       reason="dma-lane order")
                        r2, k2, w2 = rkw[:, 0, :], rkw[:, 1, :], rkw[:, 2, :]
                        e = scw.tile([128, TB], BF16, tag="wkv_e", bufs=2)
                        nc.scalar.activation(e[:], w2, AF.Exp)
                        qe = scw.tile([128, TB], BF16, tag="wkv_qe", bufs=2)
                        for cc in range(4):
                            cs = slice(128 * cc, 128 * (cc + 1))
                            nc.vector.tensor_tensor_scan(
                                qe[:, cs], e[:, cs], e[:, cs], 0.0,
                                ALU.add, ALU.bypass)
                        e2f = scw.tile([128, TB], BF16, tag="wkv_e2f", bufs=2)
                        nc.vector.scalar_tensor_tensor(
                            e2f[:], k2, c_u[:, hp:hp + 1], r2,
                            ALU.mult, ALU.mult)
                        # rt = r*exp(e-qe), kt = k*exp(qe)  (bf16)
                        nc.vector.tensor_sub(e[:], e[:], qe[:])
                        eb = scw.tile([128, TB], BF16, tag="wkv_eb", bufs=2)
                        nc.scalar.activation(eb[:], e[:], AF.Exp)
                        rt = scw.tile([128, TB], BF16, tag="wkv_rt", bufs=2)
                        nc.vector.tensor_mul(rt[:], r2, eb[:])
                        ktb = scw.tile([128, TB], BF16, tag="wkv_eb", bufs=2,
                                       name="ktb")
                        nc.scalar.activation(ktb[:], qe[:], AF.Exp)
                        kt = scw.tile([128, TB], BF16, tag="wkv_kt", bufs=2)
                        nc.vector.tensor_mul(kt[:], k2, ktb[:])
                        ypb = sc.tile([128, TB], BF16, tag="wkv_ypb",
                                      bufs=2, name="ypb")
                        for cc in range(4):
                            cs = slice(128 * cc, 128 * (cc + 1))
                            qend = qe[:, 128 * cc + 127:128 * cc + 128]
                            pl2 = sc.tile([128, 1], F32, tag="wkv_pl")
                            nc.scalar.activation(pl2[:], qend, AF.Exp,
                                                 scale=-1.0)
                            kh = sc.tile([128, 128], BF16, tag="wkv_kh")
                            nc.vector.tensor_scalar(kh[:], kt[:, cs], pl2[:],
                                                    None, ALU.mult)
                            khT = ppb(128, 128)
                            nc.tensor.transpose(khT[:], kh[:], c_id[:])
                            khTs = sc.tile([128, 128], BF16, tag="wkv_khTs")
                            nc.scalar.activation(khTs[:], khT[:], AF.Copy)
                            ypk = sc.tile([128, 128], BF16, tag="wkv_ypk")
                            for hh in range(2):
                                h64 = slice(64 * hh, 64 * (hh + 1))
                                at = pp(128, 128)
                                nc.tensor.matmul(at[:], kt[h64, cs],
                                                 rt[h64, cs],
                                                 start=True, stop=True)
                                scol = pp(128, 1)
                                nc.tensor.matmul(scol[:], e2f[h64, cs],
                                                 ones_col[h64, :],
                                                 start=True, stop=True)
                                am = sc.tile([128, 128], BF16, tag="wkv_am")
                                nc.vector.tensor_mul(am[:], at[:], c_msk[:])
                                ydg = sc.tile([128, 64], BF16, tag="wkv_ydg")
                                nc.vector.tensor_scalar(ydg[:],
                                                        v2[:, cc, h64],
                                                        scol[:], None,
                                                        ALU.mult)
                                yp = pp(128, 64)
                                nc.tensor.matmul(yp[:], am[:], v2[:, cc, h64],
                                                 start=True, stop=False)
                                nc.tensor.matmul(yp[:], rt[h64, cs],
                                                 S2[h64, :],
                                                 start=False, stop=True)
                                sps = pp(64, 64)
                                nc.tensor.matmul(sps[:], khTs[:, h64],
                                                 v2[:, cc, h64],
                                                 start=True, stop=True)
                                nc.vector.scalar_tensor_tensor(
                                    S2[h64, :], S2[h64, :], pl2[h64, :],
                                    sps[:], ALU.mult, ALU.add)
                                nc.vector.tensor_add(ypk[:, h64], yp[:],
                                                     ydg[:])
                            ypT = ppb(128, 128)
                            nc.tensor.transpose(ypT[:], ypk[:], c_id[:])
                            nc.scalar.activation(ypb[:, cs], ypT[:], AF.Copy)
                        hy = nc.sync.dma_start(dstY[j], ypb[:])
                        if hp == 1 and "yb1_first" not in last_dma:
                            last_dma["yb1_first"] = hy
                        last_dma["yb_last"] = hy
                if hp == 0:
                    nc.gpsimd.collective_compute(
                        "AllToAll", ALU.bypass, replica_groups=RG,
                        ins=[a2b0_in[:]], outs=[a2b0_out[:]])
            nc.gpsimd.collective_compute(
                "AllToAll", ALU.bypass, replica_groups=RG,
                ins=[a2b1_in[:]], outs=[a2b1_out[:]])

            # ============ y assembly + GroupNorm (token domain) ============
            yt = big.tile([128, KC, TB], BF16, tag="mA", name="yt")
            psS_e, psSq_e = pp(16, TB), pp(16, TB)
            psS_o, psSq_o = pp(16, TB), pp(16, TB)
            gst = lnp.tile([64, 2 * TB], F32, tag="ln_stats")
            gstb = lnp.tile([64, 2 * TB], BF16, tag="ln_stb")
            gvar = lnp.tile([64, TB], F32, tag="ln_work")
            for par in range(2):
                ms = list(range(par, KC, 2))
                psS = psS_e if par == 0 else psS_o
                psSq = psSq_e if par == 0 else psSq_o
                for i, m in enumerate(ms):
                    src = a2b0_out if par == 0 else a2b1_out
                    eng = nc.sync if par == 0 else nc.scalar
                    hy = eng.dma_start(yt[:, m, :], src[m // 2])
                    anchor = "yb1_first" if par == 0 else "yb_last"
                    add_dep_helper(hy.ins, last_dma[anchor].ins, sync=False,
                                   reason="dma-lane order")
                    sq = sc.tile([128, TB], BF16, tag="g1")
                    nc.scalar.activation(sq[:], yt[:, m, :], AF.Square)
                    st, sp = (i == 0), (i == 7)
                    gs = c_gsel[:, m, 16 * par:16 * (par + 1)]
                    nc.tensor.matmul(psS[:], gs, yt[:, m, :],
                                     start=st, stop=sp)
                    nc.tensor.matmul(psSq[:], gs, sq[:],
                                     start=st, stop=sp)
                rows = slice(32 * par, 32 * par + 16)
                gmean, ginv = gst[rows, 0:TB], gst[rows, TB:]
                nc.scalar.activation(gmean, psS[:], AF.Copy, scale=1.0 / 64)
                nc.scalar.activation(ginv, psSq[:], AF.Copy, scale=1.0 / 64)
                gv = gvar[rows, :]
                nc.vector.tensor_mul(gv, gst[rows, 0:TB], gst[rows, 0:TB])
                nc.vector.tensor_sub(gv, ginv, gv)
                nc.scalar.activation(ginv, gv, AF.Sqrt, bias=EPS_LNX)
                nc.vector.reciprocal(ginv, ginv)
                nc.vector.tensor_copy(gstb[rows, :], gst[rows, :])
                # gstb: [w-folded inv*mean | inv] per bsel; 3-op normalize
                nc.vector.tensor_mul(gstb[rows, 0:TB], gstb[rows, 0:TB],
                                     gstb[rows, TB:])
            for m in list(range(0, KC, 2)) + list(range(1, KC, 2)):
                bcM = pp(128, TB)
                nc.tensor.matmul(bcM[:], c_bsel[:, m, :], gstb[:, 0:TB],
                                 start=True, stop=True)
                bcI = pp(128, TB)
                nc.tensor.matmul(bcI[:], c_bsel[:, m, :], gstb[:, TB:],
                                 start=True, stop=True)
                bcMc = sc.tile([128, TB], BF16, tag="bcMc")
                nc.scalar.activation(bcMc[:], bcM[:], AF.Copy)
                bcIc = sc.tile([128, TB], BF16, tag="bcIc")
                nc.scalar.activation(bcIc[:], bcI[:], AF.Copy)
                yn = sc.tile([128, TB], BF16, tag="g1")
                nc.vector.tensor_mul(yn[:], yt[:, m, :], bcIc[:])
                nc.vector.tensor_sub(yn[:], yn[:], bcMc[:])
                nc.vector.scalar_tensor_tensor(
                    yt[:, m, :], yn[:], c_lnx[:, m, 1:2], gsb[:, m, :],
                    ALU.add, ALU.mult)

            # ============ att out proj + residual + LN2 stats ============
            x2b = big.tile([128, KC, TB], BF16, tag="xb", name="x2b")
            psA3, psA4 = pp(1, TB), pp(1, TB)
            for g4 in range(KC // 4):
                pts4 = [pp(128, TB) for _ in range(4)]
                ko = list(range(0, KC, 2)) + list(range(1, KC, 2))
                for ki, k in enumerate(ko):
                    wok = wstr.tile([128, 4, 128], BF16, tag="wok_s", bufs=3)
                    nc.sync.dma_start(wok[:],
                                      Wo_kp[k, :, 4 * g4:4 * (g4 + 1), :])
                    for mi in range(4):
                        nc.tensor.matmul(pts4[mi][:], wok[:, mi, :],
                                         yt[:, k, :], start=(ki == 0),
                                         stop=(ki == KC - 1))
                for mi in range(4):
                    m = 4 * g4 + mi
                    x2t = sc.tile([128, TB], F32, tag="g3", bufs=2)
                    xin = sc.tile([128, TB], F32, tag="g4", bufs=2)
                    nc.sync.dma_start(xin[:],
                                      xT[128 * m:128 * (m + 1), 1:TB + 1])
                    nc.vector.tensor_add(x2t[:], pts4[mi][:], xin[:])
                    nc.sync.dma_start(x2d[128 * m:128 * (m + 1), :], x2t[:])
                    nc.scalar.activation(x2b[:, m, :], x2t[:], AF.Copy)
                    sq = sc.tile([128, TB], BF16, tag="g1")
                    nc.scalar.activation(sq[:], x2b[:, m, :], AF.Square)
                    st, sp = (m == 0), (m == KC - 1)
                    nc.tensor.matmul(psA3[:], ones_col[:], x2b[:, m, :],
                                     start=st, stop=sp)
                    nc.tensor.matmul(psA4[:], ones_col[:], sq[:],
                                     start=st, stop=sp)

            # ---- ln2 normalize ----
            stats2 = lnp.tile([1, 2 * TB], F32, tag="ln_stats")
            mean2, msq2 = stats2[:, 0:TB], stats2[:, TB:]
            nc.scalar.activation(mean2[:], psA3[:], AF.Copy, scale=1.0 / C)
            nc.scalar.activation(msq2[:], psA4[:], AF.Copy, scale=1.0 / C)
            wk2 = lnp.tile([1, TB], F32, tag="ln_work")
            nc.vector.tensor_mul(wk2[:], mean2[:], mean2[:])
            nc.vector.tensor_sub(wk2[:], msq2[:], wk2[:])
            nc.scalar.activation(wk2[:], wk2[:], AF.Sqrt, bias=EPS_LN)
            nc.vector.reciprocal(wk2[:], wk2[:])
            stb2 = lnp.tile([1, 2 * TB], BF16, tag="ln_stb")
            nc.vector.tensor_copy(stb2[:, 0:TB], mean2[:])
            nc.vector.tensor_copy(stb2[:, TB:], wk2[:])
            bmp3, bip3 = pp(128, TB), pp(128, TB)
            nc.tensor.matmul(bmp3[:], ones_row[:], stb2[:, 0:TB], start=True,
                             stop=True)
            nc.tensor.matmul(bip3[:], ones_row[:], stb2[:, TB:], start=True,
                             stop=True)
            bc2 = lnp.tile([128, 2 * TB], BF16, tag="ln_bc")
            nc.vector.tensor_copy(bc2[:, 0:TB], bmp3[:])
            nc.vector.tensor_copy(bc2[:, TB:], bip3[:])
            # boundary column first so the AllGather overlaps the
            # full-width normalize pass below
            for k in range(KC):
                tb1 = sc.tile([128, 1], BF16, tag="bnd")
                nc.vector.tensor_sub(tb1[:], x2b[:, k, TB - 1:TB],
                                     bc2[:, TB - 1:TB])
                nc.vector.tensor_mul(tb1[:], tb1[:], bc2[:, 2 * TB - 1:])
                nc.vector.tensor_scalar(ht[:, k, TB:TB + 1], tb1[:],
                                        c_ln2[:, k, 0:1], c_ln2[:, k, 1:2],
                                        ALU.mult, ALU.add)
            last_dma["ag"] = nc.sync.dma_start(
                ag_in[0:1, :].rearrange("o (k p) -> p k o", p=128),
                ht[:, :, TB:TB + 1])
            nc.gpsimd.collective_compute(
                "AllGather", ALU.bypass, replica_groups=RG,
                ins=[ag_in[:]], outs=[ag_out[:]])

            for k in range(KC):
                t = sc.tile([128, TB], BF16, tag="e2")
                nc.vector.tensor_sub(t[:], x2b[:, k, :], bc2[:, 0:TB])
                nc.vector.tensor_mul(t[:], t[:], bc2[:, TB:])
                nc.vector.tensor_scalar(ht[:, k, 1:TB + 1], t[:],
                                        c_ln2[:, k, 0:1], c_ln2[:, k, 1:2],
                                        ALU.mult, ALU.add)

            for q in range(4):
                agp = sc.tile([NCORE, TB], BF16, tag="agp", bufs=1)
                ha = nc.scalar.dma_start(agp[:],
                                         ag_out[:, 512 * q:512 * (q + 1)])
                add_dep_helper(ha.ins, last_dma["ag"].ins, sync=False,
                               reason="dma-lane order")
                hp_ = pp(1, TB)
                nc.tensor.matmul(hp_[:], c_sel[:], agp[:],
                                 start=True, stop=True)
                hrow = sc.tile([1, TB], BF16, tag="hrow")
                nc.vector.tensor_copy(hrow[:], hp_[:])
                for mm in range(4):
                    m = 4 * q + mm
                    nc.sync.dma_start(ht[:, m, 0:1],
                                      hrow[0:1, 128 * mm:128 * (mm + 1)])
            for k in range(KC):
                x0 = sc.tile([128, 1], BF16, tag="bnd")
                nc.vector.tensor_sub(x0[:], ht[:, k, 0:1], ht[:, k, 1:2])
                nc.vector.scalar_tensor_tensor(
                    xx[:, k, 0:1], x0[:], c_cm[:, k, 0:1], ht[:, k, 1:2],
                    ALU.mult, ALU.add)
                nc.vector.scalar_tensor_tensor(
                    gsb[:, k, 0:1], x0[:], c_cm[:, k, 1:2], ht[:, k, 1:2],
                    ALU.mult, ALU.add)

            for k in range(KC):
                # columns 1..TB-1 need no halo; they overlap the AllGather
                xx2 = sc.tile([128, TB], BF16, tag="g1")
                nc.vector.tensor_sub(xx2[:, 1:], ht[:, k, 1:TB],
                                     ht[:, k, 2:TB + 1])
                nc.vector.scalar_tensor_tensor(
                    xx[:, k, 1:], xx2[:, 1:], c_cm[:, k, 0:1],
                    ht[:, k, 2:TB + 1], ALU.mult, ALU.add)        # xk2
                nc.vector.scalar_tensor_tensor(
                    gsb[:, k, 1:], xx2[:, 1:], c_cm[:, k, 1:2],
                    ht[:, k, 2:TB + 1], ALU.mult, ALU.add)        # xr2

            # ============ FFN ============
            kfA = big.tile([128, KC, TB], BF16, tag="mB")
            kfB = big.tile([128, KC, TB], BF16, tag="mA")
            kfC = big.tile([128, KC, TB], BF16, tag="ht")
            kfD = big.tile([128, 8, TB], BF16, tag="xb")

            def kf_view(i):
                if i < KC:
                    return kfA[:, i, :]
                if i < 32:
                    return kfB[:, i - 16, :]
                return kfC[:, i - 32, :] if i < 48 else kfD[:, i - 48, :]

            for g8 in range(KF // 8):
                pts = [pp(128, TB) for _ in range(8)]
                for k in range(KC):
                    wck = wstr.tile([128, 8, 128], BF16, tag="wck_s", bufs=3)
                    nc.sync.dma_start(wck[:],
                                      Wck_p[k, :, 8 * g8:8 * (g8 + 1), :])
                    for mi in range(8):
                        nc.tensor.matmul(pts[mi][:], wck[:, mi, :],
                                         xx[:, k, :], start=(k == 0),
                                         stop=(k == KC - 1))
                for mi in range(8):
                    mf = 8 * g8 + mi
                    rl = sc.tile([128, TB], BF16, tag="g1")
                    nc.vector.tensor_scalar(rl[:], pts[mi][:], 0.0, None,
                                            ALU.max)
                    nc.scalar.activation(kf_view(mf), rl[:], AF.Square)

            for m in range(KC):
                ptu = pp(128, TB)
                for q in range(4):
                    wcv = wstr.tile([128, 14, 128], BF16, tag="wcv_s", bufs=2)
                    nc.sync.dma_start(wcv[:],
                                      Wcv_p[m, :, q * 14:(q + 1) * 14, :])
                    for kk in range(14):
                        ki = q * 14 + kk
                        nc.tensor.matmul(ptu[:], wcv[:, kk, :], kf_view(ki),
                                         start=(ki == 0), stop=(ki == KF - 1))
                wt = wstr.tile([128, KC, 128], BF16, tag="wstream", bufs=2)
                nc.sync.dma_start(wt[:], Wp["cr"][m])
                pts = pp(128, TB)
                for k in range(KC):
                    nc.tensor.matmul(pts[:], wt[:, k, :], gsb[:, k, :],
                                     start=(k == 0), stop=(k == KC - 1))
                ssb = sc.tile([128, TB], BF16, tag="g2")
                nc.scalar.activation(ssb[:], pts[:], AF.Sigmoid)
                ot = sc.tile([128, TB], F32, tag="g3", bufs=2)
                x2in = sc.tile([128, TB], F32, tag="g4", bufs=2)
                nc.sync.dma_start(x2in[:],
                                  x2d[128 * m:128 * (m + 1), :])
                nc.vector.tensor_mul(ot[:], ptu[:], ssb[:])
                nc.vector.tensor_add(ot[:], ot[:], x2in[:])
                nc.sync.dma_start(outT[128 * m:128 * (m + 1), :], ot[:])

    nc.compile()
    return nc


_CACHE = {}


def _get_program():
    if "nc" not in _CACHE:
        _CACHE["nc"] = build_program()
    return _CACHE["nc"]


def _pret4(w):
    """(Cin, Cout) -> (Cout/128, 128, Cin/128, 128): [m,p,k,f] = w[128k+p, 128m+f]
    so each m-tile is one fully-contiguous per-partition DMA."""
    ci, co = w.shape
    return np.ascontiguousarray(
        w.reshape(ci // 128, 128, co // 128, 128).transpose(2, 1, 0, 3))


def _shard_inputs(inp):
    f32 = np.float32
    x = np.asarray(inp["x"], f32)
    bf = lambda a: np.asarray(a, f32).astype(NP_BF16)

    maa_w2 = np.asarray(inp["maa_w2"], f32)
    w2p = np.zeros((32, 5, KC, 128), f32)
    for i in range(5):
        for m in range(KC):
            w2p[:, i, m, :] = maa_w2[i][:, 128 * m:128 * (m + 1)]
    td_w2 = np.asarray(inp["td_w2"], f32)
    td2p = td_w2.reshape(64, KC, 128).copy()
    for m in range(KC):
        td2p[:, m, :] = td_w2[:, 128 * m:128 * (m + 1)]

    gsel = np.zeros((128, KC, 32), f32)
    bsel = np.zeros((64, KC, 128), f32)
    lnxw = np.asarray(inp["lnx_w"], f32)
    for p in range(128):
        for k in range(KC):
            row = 16 * (k % 2) + (k // 2) * 2 + p // 64
            gsel[p, k, row] = 1.0
            bsel[32 * (k % 2) + (k // 2) * 2 + p // 64, k, p] = \
                lnxw[128 * k + p]

    shared = {
        "ln1_wb": np.stack([inp["ln1_w"], inp["ln1_b"]], 1).astype(f32),
        "ln2_wb": np.stack([inp["ln2_w"], inp["ln2_b"]], 1).astype(f32),
        "lnx_wb": np.stack([inp["lnx_w"], inp["lnx_b"]], 1).astype(f32),
        "tm_maaT": np.asarray(inp["tm_maa"], f32).T.copy(),
        "cm_maaT": np.asarray(inp["cm_maa"], f32).T.copy(),
        "td_col": np.asarray(inp["time_decay"], f32).reshape(C, 1),
        "ident": np.eye(128, dtype=f32).astype(NP_BF16),
        "mask_su": np.triu(np.ones((128, 128), f32), 1).astype(NP_BF16),
        "gsel": gsel.reshape(128, KC * 32).astype(NP_BF16),
        "bsel": bsel.reshape(64, KC * 128).astype(NP_BF16),
        "maa_w1": bf(inp["maa_w1"]),
        "maa_w2p": w2p.astype(NP_BF16),
        "td_w1": bf(inp["td_w1"]),
        "td_w2p": td2p.astype(NP_BF16),
        "Wr_p": bf(_pret4(np.asarray(inp["Wr"], f32))),
        "Wk_p": bf(_pret4(np.asarray(inp["Wk"], f32))),
        "Wg_p": bf(_pret4(np.asarray(inp["Wg"], f32))),
        "Wo_kp": bf(np.asarray(inp["Wo"], f32).reshape(KC, 128, KC, 128)),
        "Wcr_p": bf(_pret4(np.asarray(inp["Wcr"], f32))),
        "Wv": bf(inp["Wv"]),
        "Wck_p": bf(np.asarray(inp["Wck"], f32).reshape(KC, 128, KF, 128)),
        "Wcv_p": bf(_pret4(np.asarray(inp["Wcv"], f32))),
    }
    u = np.asarray(inp["time_faaaa"], f32).reshape(C)

    in_maps = []
    for c in range(NCORE):
        b, blk = c // 4, c % 4
        ts = blk * TB
        xe = np.zeros((C, TB + 1), f32)
        xe[:, 1:] = x[b, ts:ts + TB].T
        if blk > 0:
            xe[:, 0] = x[b, ts - 1]
        ul = u[LCH * c:LCH * (c + 1)].reshape(2, 128).T.copy()
        sel = np.zeros((NCORE, 1), NP_BF16)
        if blk > 0:
            sel[c - 1, 0] = 1.0
        m = dict(shared)
        m.update({
            "xT": xe,
            "halo_mask": np.full((128, 1), 1.0 if blk > 0 else 0.0, f32),
            "sel_prev": sel,
            "u_loc": ul,
        })
        in_maps.append(m)
    return in_maps


def run(inputs, trace=False):
    nc = _get_program()
    in_maps = _shard_inputs(inputs)
    res = bass_utils.run_bass_kernel_spmd(
        nc, in_maps, core_ids=list(range(NCORE)), trace=trace)
    x = np.asarray(inputs["x"], np.float32)
    out = np.empty_like(x)
    for c in range(NCORE):
        b, blk = c // 4, c % 4
        out[b, blk * TB:(blk + 1) * TB, :] = np.asarray(
            res.results[c]["out"], np.float32).T
    return out, res.exec_time_ns


def kernel(**inputs):
    out, _ = run(inputs)
    return out


if __name__ == "__main__":
    build_program()
    print("build ok")
